# revision 1
# baseline (speedup 1.0000x reference)
"""Trainium2 Bass kernel for LocalBackwardTemporalAttention.

Sharding: data-parallel over batch B=8 across the 8 NeuronCores (one batch
element per core, no collectives). Per-core pipeline (feature-major "fm" =
[features-on-partitions, tokens-free]; token-major "tm" = transpose):

  A: LayerNorm(x) -> kv_ln (fm bf16 + tm bf16), q_ln (fm bf16)
  B: QKV projections -> q_fm, k_fm (fm), v_tm (tm)   [1/sqrt(D) folded into wq]
  C: per (t,head) attention: S=q@k^T -> softmax -> PE-transpose -> attn^T@v -> o_fm
  D: out_proj -> att_fm
  E: mlpq GEMM1(+exact gelu) -> h1q; GEMM2 in swapped (token-major-out)
     orientation + bias + kv_ln residual -> qpre_tm (f32)
  F: res_ln + ln2 (token-major) + PE-transpose -> ln2t_fm
  G: mlp GEMM1(+gelu) -> h1; GEMM2 swapped orientation + bias -> out (f32,
     token-major, written directly in final layout)

All GEMMs run with bf16 inputs / f32 PSUM accumulation (validated ~3.4e-3
absmax-relative error vs the f32 reference). LN/softmax statistics in f32.
"""

import sys

sys.path.insert(0, "/opt/trn_rl_repo")

import numpy as np
import ml_dtypes

import concourse.bass as bass
import concourse.bacc as bacc_mod
import concourse.mybir as mybir
import concourse.tile as tile
from concourse.masks import make_identity

F32 = mybir.dt.float32
BF16 = mybir.dt.bfloat16
AF = mybir.ActivationFunctionType
ALU = mybir.AluOpType
AX = mybir.AxisListType

# problem shapes (hardcoded per spec)
B, HW, NF, E, M, H = 8, 16, 196, 1024, 4096, 16
T, D = HW - 1, E // H            # 15, 64
NKV, NQ = T * NF, NF             # 2940, 196
NTOK = HW * NF                   # 3136
EPS = 1e-6
P = 128
NT = 490                         # token n-tile for fm GEMMs (2940 = 6*490)
KE, KM = E // P, M // P          # 8, 32 k-chunks
ME, MM = E // P, M // P          # m-tiles


def _ceil(a, b):
    return -(-a // b)


def _bcast_ap(handle, n):
    """[n-partition broadcast of a 1-D dram tensor] -> AP [n, len]"""
    a = handle[:]
    return bass.AP(tensor=a.tensor, offset=a.offset, ap=[[0, n], list(a.ap[0])])


def _col_ap(handle, mo):
    """1-D dram tensor (mo*128,) viewed as [128, mo] with elem (p, m) = m*128+p"""
    a = handle[:]
    return bass.AP(tensor=a.tensor, offset=a.offset, ap=[[1, P], [P, mo]])


def build_nc():
    nc = bacc_mod.Bacc(None, target_bir_lowering=False)
    t = lambda n, s, d: nc.dram_tensor(n, s, d, kind="ExternalInput")

    x = t("x", [NTOK, E], F32)
    wqT = t("wqT", [E, E], BF16)
    wkT = t("wkT", [E, E], BF16)
    wvT = t("wvT", [E, E], BF16)
    woT = t("woT", [E, E], BF16)
    w1qT = t("w1qT", [E, M], BF16)
    w2qT = t("w2qT", [M, E], BF16)
    w1T = t("w1T", [E, M], BF16)
    w2T = t("w2T", [M, E], BF16)
    bqs = t("bqs", [E], F32)
    bk = t("bk", [E], F32)
    bv = t("bv", [E], F32)
    bo = t("bo", [E], F32)
    b1q = t("b1q", [M], F32)
    b2q = t("b2q", [E], F32)
    b1 = t("b1", [M], F32)
    b2 = t("b2", [E], F32)
    gq = t("gq", [E], F32)
    bbq = t("bbq", [E], F32)
    gkv = t("gkv", [E], F32)
    bbkv = t("bbkv", [E], F32)
    gres = t("gres", [E], F32)
    bbres = t("bbres", [E], F32)
    gln2 = t("gln2", [E], F32)
    bbln2 = t("bbln2", [E], F32)
    out = nc.dram_tensor("out", [NKV, E], F32, kind="ExternalOutput")

    with tile.TileContext(nc) as tc:
        with tc.tile_pool(name="dram", bufs=1, space="DRAM") as dram, \
             tc.tile_pool(name="consts", bufs=1) as consts:
            kv_fm = dram.tile([E, NKV], BF16)
            kv_tm = dram.tile([NKV, E], BF16)
            q_fm = dram.tile([E, NQ], BF16)
            k_fm = dram.tile([E, NKV], BF16)
            v_tm = dram.tile([NKV, E], BF16)
            o_fm = dram.tile([E, NKV], BF16)
            att_fm = dram.tile([E, NKV], BF16)
            h1q_fm = dram.tile([M, NKV], BF16)
            qpre_tm = dram.tile([NKV, E], F32)
            ln2t_fm = dram.tile([E, NKV], BF16)
            h1_fm = dram.tile([M, NKV], BF16)

            ident = consts.tile([P, P], BF16)
            make_identity(nc, ident)
            epst = consts.tile([P, 1], F32)
            nc.vector.memset(epst, EPS)

            # ---------------- stage A: LN + transpose ----------------
            def ln_pass(xin_rows, gain, bias_, n_rows, fm_out, tm_out):
                with tc.tile_pool(name="ln", bufs=3) as pool, \
                     tc.tile_pool(name="lnst", bufs=4) as stp, \
                     tc.tile_pool(name="lnps", bufs=4, space="PSUM") as psp, \
                     tc.tile_pool(name="lng", bufs=1) as gp:
                    gt = gp.tile([P, E], F32, name="ln_gain")
                    bt = gp.tile([P, E], F32, name="ln_bias")
                    nc.sync.dma_start(out=gt, in_=_bcast_ap(gain, P))
                    nc.sync.dma_start(out=bt, in_=_bcast_ap(bias_, P))
                    for it in range(_ceil(n_rows, P)):
                        r0 = it * P
                        p = min(P, n_rows - r0)
                        xt = pool.tile([P, E], F32, name="ln_x")
                        nc.sync.dma_start(
                            out=xt[:p], in_=xin_rows[r0:r0 + p, :])
                        x3 = xt.rearrange("p (n f) -> p n f", n=2)
                        st = stp.tile([P, 2, 6], F32, name="ln_st")
                        for i in range(2):
                            nc.vector.bn_stats(out=st[:p, i, :], in_=x3[:p, i, :])
                        mv = stp.tile([P, 2], F32, name="ln_mv")
                        nc.vector.bn_aggr(out=mv[:p], in_=st[:p])
                        rs = stp.tile([P, 1], F32, name="ln_rs")
                        nc.scalar.activation(out=rs[:p], in_=mv[:p, 1:2],
                                             func=AF.Sqrt, bias=epst[:p])
                        nc.vector.reciprocal(out=rs[:p], in_=rs[:p])
                        y = pool.tile([P, E], BF16, name="ln_y")
                        nc.vector.tensor_scalar(
                            out=y[:p], in0=xt[:p], scalar1=mv[:p, 0:1],
                            scalar2=rs[:p], op0=ALU.subtract, op1=ALU.mult)
                        nc.vector.tensor_mul(y[:p], y[:p], gt[:p])
                        nc.vector.tensor_add(y[:p], y[:p], bt[:p])
                        if tm_out is not None:
                            nc.sync.dma_start(out=tm_out[r0:r0 + p, :], in_=y[:p])
                        tp = psp.tile([P, KE, P], BF16, name="ln_tp")
                        for e in range(KE):
                            nc.tensor.transpose(
                                out=tp[:, e, :p],
                                in_=y[:p, e * P:(e + 1) * P],
                                identity=ident[:p, :p])
                        fmt = pool.tile([P, KE, P], BF16, name="ln_fmt")
                        nc.scalar.copy(out=fmt, in_=tp)
                        dst = fm_out[:, r0:r0 + p].rearrange(
                            "(e r) c -> r e c", r=P)
                        nc.sync.dma_start(out=dst, in_=fmt[:, :, :p])

            ln_pass(x[:NKV, :], gkv, bbkv, NKV, kv_fm, kv_tm)
            ln_pass(x[NKV:, :], gq, bbq, NQ, q_fm, None)

            # ---------------- fm GEMM helper ----------------
            # out_fm[mo*P, n] = act(wT[K, mo*P].T @ x_fm[K, n] + bias_col)
            def gemm_fm(wT, x_fm_ap, n_total, kc, mo, bias_h, out_fm, act, tagp):
                with tc.tile_pool(name=tagp + "w", bufs=1) as wp, \
                     tc.tile_pool(name=tagp + "x", bufs=3) as xp, \
                     tc.tile_pool(name=tagp + "o", bufs=4) as op, \
                     tc.tile_pool(name=tagp + "ps", bufs=4, space="PSUM") as pp, \
                     tc.tile_pool(name=tagp + "b", bufs=1) as bp:
                    wsb = wp.tile([P, kc, mo * P], BF16, name=tagp + "_w")
                    for k in range(kc):
                        nc.sync.dma_start(
                            out=wsb[:, k, :],
                            in_=wT[k * P:(k + 1) * P, :])
                    bsb = bp.tile([P, mo], F32, name=tagp + "_b")
                    nc.sync.dma_start(out=bsb, in_=_col_ap(bias_h, mo))
                    nts = _ceil(n_total, NT)
                    for n in range(nts):
                        n0 = n * NT
                        w = min(NT, n_total - n0)
                        xt = xp.tile([P, kc, NT], BF16, name=tagp + "_x")
                        nc.sync.dma_start(
                            out=xt[:, :, :w],
                            in_=x_fm_ap[:, n0:n0 + w].rearrange(
                                "(k r) c -> r k c", r=P))
                        for m in range(mo):
                            ps = pp.tile([P, NT], F32, name=tagp + "_ps")
                            for k in range(kc):
                                nc.tensor.matmul(
                                    ps[:, :w],
                                    lhsT=wsb[:, k, m * P:(m + 1) * P],
                                    rhs=xt[:, k, :w],
                                    start=(k == 0), stop=(k == kc - 1))
                            ev = op.tile([P, NT], BF16, name=tagp + "_ev")
                            nc.scalar.activation(
                                out=ev[:, :w], in_=ps[:, :w], func=act,
                                bias=bsb[:, m:m + 1])
                            nc.sync.dma_start(
                                out=out_fm[m * P:(m + 1) * P, n0:n0 + w],
                                in_=ev[:, :w])

            # ---------------- stage B: QKV ----------------
            gemm_fm(wkT[:, :], kv_fm, NKV, KE, ME, bk, k_fm, AF.Identity, "kp")
            gemm_fm(wqT[:, :], q_fm, NQ, KE, ME, bqs, q_fm, AF.Identity, "qp")

            # v (token-major out): v_tm[c0:c1, :] = kv_fm[:, c0:c1].T @ wvT
            def gemm_tm(lhs_fm, kc, rhsT, n_out, bias_free_h, out_tm, resid_tm,
                        out_dt, tagp):
                nb = n_out // 512
                with tc.tile_pool(name=tagp + "w", bufs=1) as wp, \
                     tc.tile_pool(name=tagp + "h", bufs=3) as hp, \
                     tc.tile_pool(name=tagp + "o", bufs=4) as op, \
                     tc.tile_pool(name=tagp + "r", bufs=3) as rp, \
                     tc.tile_pool(name=tagp + "ps", bufs=4, space="PSUM") as pp, \
                     tc.tile_pool(name=tagp + "b", bufs=1) as bp:
                    wsb = wp.tile([P, kc, n_out], BF16, name=tagp + "_w")
                    for k in range(kc):
                        nc.sync.dma_start(
                            out=wsb[:, k, :],
                            in_=rhsT[k * P:(k + 1) * P, :])
                    bsb = bp.tile([P, n_out], F32, name=tagp + "_b")
                    nc.sync.dma_start(out=bsb, in_=_bcast_ap(bias_free_h, P))
                    for c in range(_ceil(NKV, P)):
                        c0 = c * P
                        p = min(P, NKV - c0)
                        ht = hp.tile([P, kc, P], BF16, name=tagp + "_h")
                        nc.sync.dma_start(
                            out=ht[:, :, :p],
                            in_=lhs_fm[:, c0:c0 + p].rearrange(
                                "(k r) c -> r k c", r=P))
                        rt = None
                        if resid_tm is not None:
                            rt = rp.tile([P, n_out], BF16, name=tagp + "_r")
                            nc.sync.dma_start(out=rt[:p],
                                              in_=resid_tm[c0:c0 + p, :])
                        for j in range(nb):
                            ps = pp.tile([P, 512], F32, name=tagp + "_ps")
                            for k in range(kc):
                                nc.tensor.matmul(
                                    ps[:p, :],
                                    lhsT=ht[:, k, :p],
                                    rhs=wsb[:, k, j * 512:(j + 1) * 512],
                                    start=(k == 0), stop=(k == kc - 1))
                            ev = op.tile([P, 512], out_dt, name=tagp + "_ev")
                            nc.vector.tensor_add(
                                ev[:p], ps[:p, :], bsb[:p, j * 512:(j + 1) * 512])
                            if rt is not None:
                                nc.vector.tensor_add(
                                    ev[:p], ev[:p], rt[:p, j * 512:(j + 1) * 512])
                            nc.sync.dma_start(
                                out=out_tm[c0:c0 + p, j * 512:(j + 1) * 512],
                                in_=ev[:p])

            gemm_tm(kv_fm, KE, wvT[:, :], E, bv, v_tm, None, BF16, "vp")

            # ---------------- stage C: attention ----------------
            with tc.tile_pool(name="cq", bufs=1) as cqp, \
                 tc.tile_pool(name="ckv", bufs=4) as ckv, \
                 tc.tile_pool(name="cat", bufs=3) as cat, \
                 tc.tile_pool(name="cst", bufs=4) as cst, \
                 tc.tile_pool(name="co", bufs=3) as cop, \
                 tc.tile_pool(name="cps", bufs=2, space="PSUM") as cps, \
                 tc.tile_pool(name="cpt", bufs=2, space="PSUM") as cpt, \
                 tc.tile_pool(name="cpo", bufs=2, space="PSUM") as cpo:
                qsb = cqp.tile([P, KE, NQ], BF16)
                nc.sync.dma_start(
                    out=qsb, in_=q_fm[:, :].rearrange("(e r) c -> r e c", r=P))
                nch = [(0, P), (P, NQ - P)]           # n/m chunks: 128 + 68
                for t_ in range(T):
                    t0 = t_ * NF
                    osb = cop.tile([P, KE, NQ], BF16, name="c_osb")
                    for hp in range(H // 2):
                      ksb = ckv.tile([P, NQ], BF16, name="c_k")
                      nc.sync.dma_start(
                          out=ksb,
                          in_=k_fm[hp * P:(hp + 1) * P, t0:t0 + NF])
                      vsb = ckv.tile([P, 2, P], BF16, name="c_v")
                      for j, (m0, mj) in enumerate(nch):
                          nc.sync.dma_start(
                              out=vsb[:mj, j, :],
                              in_=v_tm[t0 + m0:t0 + m0 + mj,
                                       hp * P:(hp + 1) * P])
                      for pi in range(2):
                        d0 = pi * D
                        # S = q @ k^T  (scale already folded into wq)
                        ps = cps.tile([P, 2, 512], F32, name="c_ps")
                        for j, (n0, pn) in enumerate(nch):
                            nc.tensor.matmul(
                                ps[:pn, j, :NQ],
                                lhsT=qsb[d0:d0 + D, hp, n0:n0 + pn],
                                rhs=ksb[d0:d0 + D, :],
                                start=True, stop=True)
                        nm = cst.tile([P, 2], F32, name="c_nm")
                        nc.vector.reduce_max(
                            out=nm, in_=ps[:, :, :NQ], axis=AX.X, negate=True)
                        asb = cat.tile([P, 2, NQ], BF16, name="c_asb")
                        sm = cst.tile([P, 2], F32, name="c_sm")
                        for j, (n0, pn) in enumerate(nch):
                            nc.scalar.activation(
                                out=asb[:pn, j, :], in_=ps[:pn, j, :NQ],
                                func=AF.Exp, bias=nm[:pn, j:j + 1],
                                accum_out=sm[:pn, j:j + 1])
                        rc = cst.tile([P, 2], F32, name="c_rc")
                        nc.vector.reciprocal(out=rc, in_=sm)
                        for j, (n0, pn) in enumerate(nch):
                            nc.vector.tensor_scalar_mul(
                                asb[:pn, j, :], in0=asb[:pn, j, :],
                                scalar1=rc[:pn, j:j + 1])
                        # transpose attn -> attnT
                        pt = cpt.tile([P, 2, NQ], BF16, name="c_pt")
                        for jn, (n0, pn) in enumerate(nch):
                            for jm, (m0, mj) in enumerate(nch):
                                nc.tensor.transpose(
                                    out=pt[:mj, jm, n0:n0 + pn],
                                    in_=asb[:pn, jn, m0:m0 + mj],
                                    identity=ident[:pn, :pn])
                        atT = cat.tile([P, 2, NQ], BF16, name="c_atT")
                        nc.scalar.copy(out=atT, in_=pt)
                        # o^T[d, n] = sum_m v[m, d] * attnT[m, n]
                        po = cpo.tile([D, NQ], F32, name="c_po")
                        for jm, (m0, mj) in enumerate(nch):
                            nc.tensor.matmul(
                                po[:, :],
                                lhsT=vsb[:mj, jm, d0:d0 + D],
                                rhs=atT[:mj, jm, :],
                                start=(jm == 0), stop=(jm == 1))
                        nc.scalar.copy(out=osb[d0:d0 + D, hp, :],
                                       in_=po)
                    nc.sync.dma_start(
                        out=o_fm[:, t0:t0 + NF].rearrange(
                            "(e r) c -> r e c", r=P),
                        in_=osb)

            # ---------------- stage D: out_proj ----------------
            gemm_fm(woT[:, :], o_fm, NKV, KE, ME, bo, att_fm, AF.Identity, "op")

            # ---------------- stage E: mlpq ----------------
            gemm_fm(w1qT[:, :], att_fm, NKV, KE, MM, b1q, h1q_fm, AF.Gelu, "e1")
            gemm_tm(h1q_fm, KM, w2qT[:, :], E, b2q, qpre_tm, kv_tm, F32, "e2")

            # ---------------- stage F: res_ln + ln2 + transpose ----------------
            with tc.tile_pool(name="f", bufs=3) as fp, \
                 tc.tile_pool(name="fst", bufs=4) as fst, \
                 tc.tile_pool(name="fps", bufs=4, space="PSUM") as fps, \
                 tc.tile_pool(name="fg", bufs=1) as fg:
                g1 = fg.tile([P, E], F32)
                bb1 = fg.tile([P, E], F32)
                g2 = fg.tile([P, E], F32)
                bb2 = fg.tile([P, E], F32)
                nc.sync.dma_start(out=g1, in_=_bcast_ap(gres, P))
                nc.sync.dma_start(out=bb1, in_=_bcast_ap(bbres, P))
                nc.sync.dma_start(out=g2, in_=_bcast_ap(gln2, P))
                nc.sync.dma_start(out=bb2, in_=_bcast_ap(bbln2, P))
                for it in range(_ceil(NKV, P)):
                    r0 = it * P
                    p = min(P, NKV - r0)
                    xt = fp.tile([P, E], F32, name="f_x")
                    nc.sync.dma_start(out=xt[:p], in_=qpre_tm[r0:r0 + p, :])
                    cur = xt
                    for li, (gg, bb) in enumerate(((g1, bb1), (g2, bb2))):
                        x3 = cur.rearrange("p (n f) -> p n f", n=2)
                        st = fst.tile([P, 2, 6], F32, name="f_st")
                        for i in range(2):
                            nc.vector.bn_stats(out=st[:p, i, :], in_=x3[:p, i, :])
                        mv = fst.tile([P, 2], F32, name="f_mv")
                        nc.vector.bn_aggr(out=mv[:p], in_=st[:p])
                        rs = fst.tile([P, 1], F32, name="f_rs")
                        nc.scalar.activation(out=rs[:p], in_=mv[:p, 1:2],
                                             func=AF.Sqrt, bias=epst[:p])
                        nc.vector.reciprocal(out=rs[:p], in_=rs[:p])
                        y = fp.tile([P, E], F32 if li == 0 else BF16,
                                    name=f"f_y{li}")
                        nc.vector.tensor_scalar(
                            out=y[:p], in0=cur[:p], scalar1=mv[:p, 0:1],
                            scalar2=rs[:p], op0=ALU.subtract, op1=ALU.mult)
                        nc.vector.tensor_mul(y[:p], y[:p], gg[:p])
                        nc.vector.tensor_add(y[:p], y[:p], bb[:p])
                        cur = y
                    tp = fps.tile([P, KE, P], BF16, name="f_tp")
                    for e in range(KE):
                        nc.tensor.transpose(
                            out=tp[:, e, :p],
                            in_=cur[:p, e * P:(e + 1) * P],
                            identity=ident[:p, :p])
                    fmt = fp.tile([P, KE, P], BF16, name="f_fmt")
                    nc.scalar.copy(out=fmt, in_=tp)
                    nc.sync.dma_start(
                        out=ln2t_fm[:, r0:r0 + p].rearrange(
                            "(e r) c -> r e c", r=P),
                        in_=fmt[:, :, :p])

            # ---------------- stage G: mlp ----------------
            gemm_fm(w1T[:, :], ln2t_fm, NKV, KE, MM, b1, h1_fm, AF.Gelu, "g1")
            gemm_tm(h1_fm, KM, w2T[:, :], E, b2, out[:, :], None, F32, "g2")

    nc.compile()
    return nc


_NC = None


def _get_nc():
    global _NC
    if _NC is None:
        _NC = build_nc()
    return _NC


def _prep_in_maps(inputs):
    f32 = lambda a: np.ascontiguousarray(np.asarray(a, dtype=np.float32))
    bf = lambda a: np.ascontiguousarray(
        np.asarray(a, dtype=np.float32).astype(ml_dtypes.bfloat16))
    x = f32(inputs["inputs"])                       # (B,HW,NF,E)
    ipw = f32(inputs["in_proj_w"])
    ipb = f32(inputs["in_proj_b"])
    wq, wk, wv = ipw[:E], ipw[E:2 * E], ipw[2 * E:]
    bq, bk_, bv_ = ipb[:E], ipb[E:2 * E], ipb[2 * E:]
    s = 1.0 / np.sqrt(np.float32(D))
    shared = {
        "wqT": bf(wq.T * s), "wkT": bf(wk.T), "wvT": bf(wv.T),
        "woT": bf(f32(inputs["out_proj_w"]).T),
        "w1qT": bf(f32(inputs["mlpq_w1"]).T),
        "w2qT": bf(f32(inputs["mlpq_w2"]).T),
        "w1T": bf(f32(inputs["mlp_w1"]).T),
        "w2T": bf(f32(inputs["mlp_w2"]).T),
        "bqs": f32(bq * s), "bk": f32(bk_), "bv": f32(bv_),
        "bo": f32(inputs["out_proj_b"]),
        "b1q": f32(inputs["mlpq_b1"]), "b2q": f32(inputs["mlpq_b2"]),
        "b1": f32(inputs["mlp_b1"]), "b2": f32(inputs["mlp_b2"]),
        "gq": f32(inputs["ln_q_g"]), "bbq": f32(inputs["ln_q_b"]),
        "gkv": f32(inputs["ln_kv_g"]), "bbkv": f32(inputs["ln_kv_b"]),
        "gres": f32(inputs["res_ln_g"]), "bbres": f32(inputs["res_ln_b"]),
        "gln2": f32(inputs["ln2_g"]), "bbln2": f32(inputs["ln2_b"]),
    }
    return [dict(shared, x=np.ascontiguousarray(x[b].reshape(NTOK, E)))
            for b in range(B)]


def _run(inputs, trace=False):
    from concourse.bass_utils import run_bass_kernel_spmd
    nc = _get_nc()
    in_maps = _prep_in_maps(inputs)
    res = run_bass_kernel_spmd(nc, in_maps, core_ids=list(range(B)),
                               trace=trace)
    outs = np.stack([r["out"].reshape(T, NF, E) for r in res.results])
    return outs, res


def kernel(**inputs) -> np.ndarray:
    outs, _ = _run(inputs, trace=False)
    return outs



# revision 2
# speedup vs baseline: 1.0357x; 1.0357x over previous
"""Trainium2 Bass kernel for LocalBackwardTemporalAttention (optimized v2).

Data-parallel over batch B=8 (one element per core, no collectives).
Single fused on-chip pipeline; SBUF-resident intermediates with
tag-chained slot reuse (two sequential resident pools res1/res2):

  A: LN(x) -> kv_fm (SBUF fm bf16), kv_tm -> DRAM; q_ln -> SBUF
  B: QKV GEMMs from kv_fm -> k_fm, v_res (per-t token-major), q_fm
  C: attention per (hp, t-pair, pi): S = q@k^T (no max-sub) -> exp ->
     sum/recip/scale -> PE-transpose -> attnT@v -> o_fm (SBUF)
  D: out_proj o_fm -> att_fm (SBUF)
  E1: mlpq GEMM1 (+gelu), M-halved weights -> h1q (DRAM)
  --- res1 -> res2 ---
  E2+F: per 128-token chunk: GEMM2 swapped + b2q + kv_tm residual
        -> res_ln -> ln2 -> PE-transpose -> ln2t_fm (SBUF)
  G1: mlp GEMM1 (+gelu), M-halved -> h1 (DRAM)
  G2: GEMM2 swapped + b2 -> out (DRAM, token-major f32)

All GEMMs bf16 in / f32 PSUM accum. DRAM round trips only for kv_tm,
h1q, h1. DMA transfers are 0.125-4 MB (vs ~1100 tiny DMAs in v1).
"""

import sys

sys.path.insert(0, "/opt/trn_rl_repo")

import numpy as np
import ml_dtypes

import concourse.bass as bass
import concourse.bacc as bacc_mod
import concourse.mybir as mybir
import concourse.tile as tile
from concourse.masks import make_identity

F32 = mybir.dt.float32
BF16 = mybir.dt.bfloat16
AF = mybir.ActivationFunctionType
ALU = mybir.AluOpType
AX = mybir.AxisListType

B, HW, NF, E, M, H = 8, 16, 196, 1024, 4096, 16
T, D = HW - 1, E // H            # 15, 64
NKV, NQ, NTOK = T * NF, NF, HW * NF
EPS = 1e-6
P = 128
NT = 490                         # fm GEMM token tile (2940 = 6*490)
KE, KM = E // P, M // P          # 8, 32
EH = E // 2                      # 512
NTILES = NKV // NT               # 6
MH = M // 2                      # 2048


def _ceil(a, b):
    return -(-a // b)


def _bcast_ap(handle, n):
    a = handle[:]
    return bass.AP(tensor=a.tensor, offset=a.offset, ap=[[0, n], list(a.ap[0])])


def _col_ap(handle, mo):
    a = handle[:]
    return bass.AP(tensor=a.tensor, offset=a.offset, ap=[[1, P], [P, mo]])


def build_nc():
    nc = bacc_mod.Bacc(None, target_bir_lowering=False)
    t = lambda n, s, d: nc.dram_tensor(n, s, d, kind="ExternalInput")

    x = t("x", [NTOK, E], F32)
    wqT = t("wqT", [E, E], BF16)
    wkT = t("wkT", [E, E], BF16)
    wvT = t("wvT", [E, E], BF16)
    woT = t("woT", [E, E], BF16)
    w1qT = t("w1qT", [E, M], BF16)
    w2qT = t("w2qT", [M, E], BF16)
    w1T = t("w1T", [E, M], BF16)
    w2T = t("w2T", [M, E], BF16)
    bqs = t("bqs", [E], F32)
    bk = t("bk", [E], F32)
    bv = t("bv", [E], F32)
    bo = t("bo", [E], F32)
    b1q = t("b1q", [M], F32)
    b2q = t("b2q", [E], F32)
    b1 = t("b1", [M], F32)
    b2 = t("b2", [E], F32)
    gq = t("gq", [E], F32)
    bbq = t("bbq", [E], F32)
    gkv = t("gkv", [E], F32)
    bbkv = t("bbkv", [E], F32)
    gres = t("gres", [E], F32)
    bbres = t("bbres", [E], F32)
    gln2 = t("gln2", [E], F32)
    bbln2 = t("bbln2", [E], F32)
    out = nc.dram_tensor("out", [NKV, E], F32, kind="ExternalOutput")

    with tile.TileContext(nc) as tc:
        with tc.tile_pool(name="dram", bufs=1, space="DRAM") as dram, \
             tc.tile_pool(name="consts", bufs=1) as consts:
            kv_tm = dram.tile([NKV, E], BF16)
            h1q_nt = dram.tile([NTILES, P, KM, NT], BF16)
            h1_nt = dram.tile([NTILES, P, KM, NT], BF16)

            ident = consts.tile([P, P], BF16)
            make_identity(nc, ident)
            epst = consts.tile([P, 1], F32)
            nc.vector.memset(epst, EPS)

            # =================== res1: stages A..E1 ===================
            with tc.tile_pool(name="res1", bufs=1) as res1:
                # tagA: kv_fm -> o_fm          (47 KB/part)
                # tagB: k_fm -> att_fm         (47 KB)
                # tagC: v_res -> w1q halves    (60 KB)
                # tagD: qln_fm -> wo           (16 KB)
                # tagE: q_fm                   (3 KB)
                kv_fm = res1.tile([P, KE, NKV], BF16, tag="tagA", name="kv_fm")
                qln_fm = res1.tile([P, KE, NQ], BF16, tag="tagD",
                                   name="qln_fm")

                # ---------------- stage A: LN + transpose ----------------
                def ln_rows(xrows, gain, bias_, n_rows, fm_dst, tm_dst):
                    with tc.tile_pool(name="lnx", bufs=2) as xp, \
                         tc.tile_pool(name="lnst", bufs=4) as stp, \
                         tc.tile_pool(name="lnps", bufs=2, space="PSUM") as psp, \
                         tc.tile_pool(name="lng", bufs=1) as gp:
                        gt = gp.tile([P, E], BF16, name="ln_g")
                        bt = gp.tile([P, E], BF16, name="ln_b")
                        nc.gpsimd.dma_start(out=gt, in_=_bcast_ap(gain, P))
                        nc.gpsimd.dma_start(out=bt, in_=_bcast_ap(bias_, P))
                        for it in range(_ceil(n_rows, P)):
                            r0 = it * P
                            p = min(P, n_rows - r0)
                            xt = xp.tile([P, E], F32, name="ln_x")
                            nc.sync.dma_start(out=xt[:p],
                                              in_=xrows[r0:r0 + p, :])
                            x3 = xt.rearrange("p (n f) -> p n f", n=2)
                            st = stp.tile([P, 2, 6], F32, name="ln_st")
                            for i in range(2):
                                nc.vector.bn_stats(out=st[:p, i, :],
                                                   in_=x3[:p, i, :])
                            mv = stp.tile([P, 2], F32, name="ln_mv")
                            nc.vector.bn_aggr(out=mv[:p], in_=st[:p])
                            rs = stp.tile([P, 1], F32, name="ln_rs")
                            nc.scalar.activation(out=rs[:p], in_=mv[:p, 1:2],
                                                 func=AF.Sqrt, bias=epst[:p])
                            nc.vector.reciprocal(out=rs[:p], in_=rs[:p])
                            y = xp.tile([P, E], BF16, name="ln_y")
                            nc.vector.tensor_scalar(
                                out=y[:p], in0=xt[:p], scalar1=mv[:p, 0:1],
                                scalar2=rs[:p], op0=ALU.subtract,
                                op1=ALU.mult)
                            nc.vector.tensor_mul(y[:p], y[:p], gt[:p])
                            nc.vector.tensor_add(y[:p], y[:p], bt[:p])
                            if tm_dst is not None:
                                nc.sync.dma_start(out=tm_dst[r0:r0 + p, :],
                                                  in_=y[:p])
                            tp = psp.tile([P, KE, P], BF16, name="ln_tp")
                            for e in range(KE):
                                nc.tensor.transpose(
                                    out=tp[:, e, :p],
                                    in_=y[:p, e * P:(e + 1) * P],
                                    identity=ident[:p, :p])
                            nc.scalar.copy(out=fm_dst[:, :, r0:r0 + p],
                                           in_=tp[:, :, :p])

                ln_rows(x[:NKV, :], gkv, bbkv, NKV, kv_fm, kv_tm)
                ln_rows(x[NKV:, :], gq, bbq, NQ, qln_fm, None)

                # ---------------- stage B: QKV ----------------
                k_fm = res1.tile([P, KE, NKV], BF16, tag="tagB", name="k_fm")
                v_res = res1.tile([P, T, 2, E], BF16, tag="tagC",
                                  name="v_res")
                q_fm = res1.tile([P, KE, NQ], BF16, tag="tagE", name="q_fm")

                def fm_gemm(wT_h, bias_h, src_fm, n_total, dst_fm, act, mo,
                            kc, tag):
                    with tc.tile_pool(name=tag + "w", bufs=1) as wp, \
                         tc.tile_pool(name=tag + "b", bufs=1) as bp, \
                         tc.tile_pool(name=tag + "ps", bufs=4,
                                      space="PSUM") as pp:
                        wsb = wp.tile([P, kc, mo * P], BF16, name=tag + "_w")
                        for k in range(kc):
                            nc.sync.dma_start(out=wsb[:, k, :],
                                              in_=wT_h[k * P:(k + 1) * P, :])
                        bsb = bp.tile([P, mo], F32, name=tag + "_b")
                        nc.sync.dma_start(out=bsb, in_=_col_ap(bias_h, mo))
                        for n in range(_ceil(n_total, NT)):
                            n0 = n * NT
                            w = min(NT, n_total - n0)
                            for m in range(mo):
                                ps = pp.tile([P, NT], F32, name=tag + "_ps")
                                for k in range(kc):
                                    nc.tensor.matmul(
                                        ps[:, :w],
                                        lhsT=wsb[:, k, m * P:(m + 1) * P],
                                        rhs=src_fm[:, k, n0:n0 + w],
                                        start=(k == 0), stop=(k == kc - 1))
                                nc.scalar.activation(
                                    out=dst_fm[:, m, n0:n0 + w],
                                    in_=ps[:, :w], func=act,
                                    bias=bsb[:, m:m + 1])

                fm_gemm(wkT, bk, kv_fm, NKV, k_fm, AF.Identity, KE, KE, "bk")
                fm_gemm(wqT, bqs, qln_fm, NQ, q_fm, AF.Identity, KE, KE,
                        "bq")

                # v: token-major per (t, half) via swapped orientation
                with tc.tile_pool(name="vw", bufs=1) as vwp, \
                     tc.tile_pool(name="vb", bufs=1) as vbp, \
                     tc.tile_pool(name="vps", bufs=3, space="PSUM") as vpp:
                    wsb = vwp.tile([P, KE, E], BF16, name="v_w")
                    for k in range(KE):
                        nc.sync.dma_start(out=wsb[:, k, :],
                                          in_=wvT[k * P:(k + 1) * P, :])
                    bvb = vbp.tile([P, E], F32, name="v_b")
                    nc.sync.dma_start(out=bvb, in_=_bcast_ap(bv, P))
                    for t_ in range(T):
                        for half in range(2):
                            r0 = t_ * NF + half * P
                            pr = P if half == 0 else NF - P
                            ps = vpp.tile([P, E], F32, name="v_ps")
                            for j in range(2):
                                for k in range(KE):
                                    nc.tensor.matmul(
                                        ps[:pr, j * EH:(j + 1) * EH],
                                        lhsT=kv_fm[:, k, r0:r0 + pr],
                                        rhs=wsb[:, k, j * EH:(j + 1) * EH],
                                        start=(k == 0), stop=(k == KE - 1))
                            nc.vector.tensor_add(v_res[:pr, t_, half, :],
                                                 ps[:pr, :], bvb[:pr, :])

                # ---------------- stage C: attention ----------------
                o_fm = res1.tile([P, KE, NKV], BF16, tag="tagA", name="o_fm")

                nch = [(0, P), (P, NQ - P)]
                tpairs = [(t0, min(2, T - t0)) for t0 in range(0, T, 2)]
                with tc.tile_pool(name="cat", bufs=3) as cat, \
                     tc.tile_pool(name="cst", bufs=4) as cst, \
                     tc.tile_pool(name="cps", bufs=2, space="PSUM") as cps, \
                     tc.tile_pool(name="cpt", bufs=2, space="PSUM") as cpt, \
                     tc.tile_pool(name="cpo", bufs=2, space="PSUM") as cpo:
                    for hp in range(H // 2):
                        for (tb, tn) in tpairs:
                            po = cpo.tile([P, 2, NQ], F32, name="c_po")
                            for pi in range(2):
                                d0 = pi * D
                                asbs = []
                                rc = cst.tile([P, 2, 2], F32, name="c_rc")
                                sm = cst.tile([P, 2, 2], F32, name="c_sm")
                                for j, (n0, pn) in enumerate(nch):
                                    ps = cps.tile([P, 2 * NQ], F32,
                                                  name=f"c_ps{j}")
                                    nc.tensor.matmul(
                                        ps[:pn, :tn * NQ],
                                        lhsT=q_fm[d0:d0 + D, hp, n0:n0 + pn],
                                        rhs=k_fm[d0:d0 + D, hp,
                                                 tb * NF:(tb + tn) * NF],
                                        start=True, stop=True)
                                    asb = cat.tile([P, 2, NQ], BF16,
                                                   name=f"c_asb{j}")
                                    nc.scalar.activation(
                                        out=asb[:pn, :tn, :].rearrange(
                                            "p t n -> p (t n)"),
                                        in_=ps[:pn, :tn * NQ], func=AF.Exp)
                                    nc.vector.reduce_sum(
                                        out=sm[:pn, j, :tn],
                                        in_=asb[:pn, :tn, :], axis=AX.X)
                                    asbs.append(asb)
                                nc.vector.reciprocal(out=rc, in_=sm)
                                for j, (n0, pn) in enumerate(nch):
                                    for ti in range(tn):
                                        nc.vector.tensor_scalar_mul(
                                            asbs[j][:pn, ti, :],
                                            in0=asbs[j][:pn, ti, :],
                                            scalar1=rc[:pn, j, ti:ti + 1])
                                pt = cpt.tile([P, 2, 2, NQ], BF16,
                                              name="c_pt")
                                for ti in range(tn):
                                    for jn, (n0, pn) in enumerate(nch):
                                        for jm, (m0, mj) in enumerate(nch):
                                            nc.tensor.transpose(
                                                out=pt[:mj, ti, jm,
                                                       n0:n0 + pn],
                                                in_=asbs[jn][:pn, ti,
                                                             m0:m0 + mj],
                                                identity=ident[:pn, :pn])
                                atT = cat.tile([P, 2, 2, NQ], BF16,
                                               name="c_atT")
                                nc.scalar.copy(out=atT[:, :tn],
                                               in_=pt[:, :tn])
                                for ti in range(tn):
                                    for jm, (m0, mj) in enumerate(nch):
                                        nc.tensor.matmul(
                                            po[d0:d0 + D, ti, :],
                                            lhsT=v_res[:mj, tb + ti, jm,
                                                       hp * P + d0:
                                                       hp * P + d0 + D],
                                            rhs=atT[:mj, ti, jm, :],
                                            start=(jm == 0), stop=(jm == 1))
                            nc.scalar.copy(
                                out=o_fm[:, hp, tb * NF:(tb + tn) * NF],
                                in_=po[:, :tn, :])

                # ---------------- stage D: out_proj ----------------
                att_fm = res1.tile([P, KE, NKV], BF16, tag="tagB",
                                   name="att_fm")
                wo_sb = res1.tile([P, KE, E], BF16, tag="tagD", name="wo_sb")
                with tc.tile_pool(name="dob", bufs=1) as dbp, \
                     tc.tile_pool(name="dops", bufs=4, space="PSUM") as dpp:
                    for k in range(KE):
                        nc.sync.dma_start(out=wo_sb[:, k, :],
                                          in_=woT[k * P:(k + 1) * P, :])
                    bsb = dbp.tile([P, KE], F32, name="do_b")
                    nc.sync.dma_start(out=bsb, in_=_col_ap(bo, KE))
                    for n in range(NTILES):
                        n0 = n * NT
                        for m in range(KE):
                            ps = dpp.tile([P, NT], F32, name="do_ps")
                            for k in range(KE):
                                nc.tensor.matmul(
                                    ps[:, :],
                                    lhsT=wo_sb[:, k, m * P:(m + 1) * P],
                                    rhs=o_fm[:, k, n0:n0 + NT],
                                    start=(k == 0), stop=(k == KE - 1))
                            nc.scalar.activation(
                                out=att_fm[:, m, n0:n0 + NT], in_=ps[:, :],
                                func=AF.Identity, bias=bsb[:, m:m + 1])

                # ---------------- stage E1: mlpq GEMM1 ----------------
                def mlp_g1(wT_h, bias_h, src_fm, dst_nt, wpool, wtag, tag):
                    with tc.tile_pool(name=tag + "b", bufs=1) as bp, \
                         tc.tile_pool(name=tag + "bn", bufs=3) as bnp, \
                         tc.tile_pool(name=tag + "ps", bufs=4,
                                      space="PSUM") as pp:
                        bsb = bp.tile([P, KM], F32, name=tag + "_b")
                        nc.sync.dma_start(out=bsb, in_=_col_ap(bias_h, KM))
                        for mh in range(2):
                            wsb = wpool.tile([P, KE, MH], BF16, tag=wtag,
                                             name=tag + f"_w{mh}")
                            for k in range(KE):
                                nc.sync.dma_start(
                                    out=wsb[:, k, :],
                                    in_=wT_h[k * P:(k + 1) * P,
                                             mh * MH:(mh + 1) * MH])
                            for n in range(NTILES):
                                n0 = n * NT
                                for m in range(KM // 2):
                                    mi = mh * (KM // 2) + m
                                    ps = pp.tile([P, NT], F32,
                                                 name=tag + "_ps")
                                    for k in range(KE):
                                        nc.tensor.matmul(
                                            ps[:, :],
                                            lhsT=wsb[:, k,
                                                     m * P:(m + 1) * P],
                                            rhs=src_fm[:, k, n0:n0 + NT],
                                            start=(k == 0),
                                            stop=(k == KE - 1))
                                    hb = bnp.tile([P, NT], BF16,
                                                  name=tag + "_hb")
                                    nc.scalar.activation(
                                        out=hb, in_=ps[:, :], func=AF.Gelu,
                                        bias=bsb[:, mi:mi + 1])
                                    nc.sync.dma_start(
                                        out=dst_nt[n][:, mi, :], in_=hb)

                mlp_g1(w1qT, b1q, att_fm, h1q_nt, res1, "tagC", "e1")

            # =================== res2: stages E2..G2 ===================
            with tc.tile_pool(name="res2", bufs=1) as res2:
                # tagH: ln2t_fm (47 KB); tagW2: w2q -> w2 (64 KB)
                # tagW1: w1 halves (32 KB); quarters via scoped pools
                ln2t_fm = res2.tile([P, KE, NKV], BF16, tag="tagH",
                                    name="ln2t_fm")

                def ln_tm(stp, src, dst, gg, bb, p):
                    x3 = src.rearrange("p (n f) -> p n f", n=2)
                    st = stp.tile([P, 2, 6], F32, name="f_st")
                    for i in range(2):
                        nc.vector.bn_stats(out=st[:p, i, :], in_=x3[:p, i, :])
                    mv = stp.tile([P, 2], F32, name="f_mv")
                    nc.vector.bn_aggr(out=mv[:p], in_=st[:p])
                    rs = stp.tile([P, 1], F32, name="f_rs")
                    nc.scalar.activation(out=rs[:p], in_=mv[:p, 1:2],
                                         func=AF.Sqrt, bias=epst[:p])
                    nc.vector.reciprocal(out=rs[:p], in_=rs[:p])
                    nc.vector.tensor_scalar(
                        out=dst[:p], in0=src[:p], scalar1=mv[:p, 0:1],
                        scalar2=rs[:p], op0=ALU.subtract, op1=ALU.mult)
                    nc.vector.tensor_mul(dst[:p], dst[:p], gg[:p])
                    nc.vector.tensor_add(dst[:p], dst[:p], bb[:p])

                def mlp_g2(w_sb, src_nt, tag, epilogue):
                    with tc.tile_pool(name=tag + "ps", bufs=2,
                                      space="PSUM") as pp:
                        for n in range(NTILES):
                            hq = res2.tile([P, KM, NT], BF16, tag="tagW1",
                                           name=tag + "_h")
                            for qtr in range(4):
                                nc.sync.dma_start(
                                    out=hq[:, qtr * 8:(qtr + 1) * 8, :],
                                    in_=src_nt[n][:, qtr * 8:(qtr + 1) * 8,
                                                  :])
                            subs = [(0, P), (P, P), (2 * P, P),
                                    (3 * P, NT - 3 * P)]
                            for (s0, pr) in subs:
                                ps = pp.tile([P, E], F32, name=tag + "_ps")
                                for j in range(2):
                                    for k in range(KM):
                                        nc.tensor.matmul(
                                            ps[:pr, j * EH:(j + 1) * EH],
                                            lhsT=hq[:, k, s0:s0 + pr],
                                            rhs=w_sb[:, k,
                                                     j * EH:(j + 1) * EH],
                                            start=(k == 0),
                                            stop=(k == KM - 1))
                                epilogue(ps, n * NT + s0, pr)

                # ---------------- stage E2 + F ----------------
                w2q_sb = res2.tile([P, KM, E], BF16, tag="tagW2",
                                   name="w2q_sb")
                for k in range(KM):
                    nc.sync.dma_start(out=w2q_sb[:, k, :],
                                      in_=w2qT[k * P:(k + 1) * P, :])
                with tc.tile_pool(name="fg", bufs=1) as fg, \
                     tc.tile_pool(name="fst", bufs=4) as fst, \
                     tc.tile_pool(name="fw", bufs=2) as fwp, \
                     tc.tile_pool(name="fkv", bufs=2) as fkv, \
                     tc.tile_pool(name="ftp", bufs=2, space="PSUM") as ftp:
                    b2qb = fg.tile([P, E], F32, name="f_b2q")
                    g1t = fg.tile([P, E], BF16, name="f_g1")
                    b1t = fg.tile([P, E], BF16, name="f_b1")
                    g2t = fg.tile([P, E], BF16, name="f_g2")
                    b2t = fg.tile([P, E], BF16, name="f_b2")
                    nc.sync.dma_start(out=b2qb, in_=_bcast_ap(b2q, P))
                    nc.gpsimd.dma_start(out=g1t, in_=_bcast_ap(gres, P))
                    nc.gpsimd.dma_start(out=b1t, in_=_bcast_ap(bbres, P))
                    nc.gpsimd.dma_start(out=g2t, in_=_bcast_ap(gln2, P))
                    nc.gpsimd.dma_start(out=b2t, in_=_bcast_ap(bbln2, P))

                    def e2_epilogue(ps, r0, pr):
                        kvt = fkv.tile([P, E], BF16, name="f_kv")
                        nc.sync.dma_start(out=kvt[:pr],
                                          in_=kv_tm[r0:r0 + pr, :])
                        qs = fwp.tile([P, E], F32, name="f_qs")
                        nc.vector.tensor_add(qs[:pr], ps[:pr, :], b2qb[:pr])
                        nc.vector.tensor_add(qs[:pr], qs[:pr], kvt[:pr])
                        y1 = fwp.tile([P, E], F32, name="f_y1")
                        ln_tm(fst, qs, y1, g1t, b1t, pr)
                        y2 = fwp.tile([P, E], BF16, name="f_y2")
                        ln_tm(fst, y1, y2, g2t, b2t, pr)
                        tp = ftp.tile([P, KE, P], BF16, name="f_tp")
                        for e in range(KE):
                            nc.tensor.transpose(
                                out=tp[:, e, :pr],
                                in_=y2[:pr, e * P:(e + 1) * P],
                                identity=ident[:pr, :pr])
                        nc.scalar.copy(out=ln2t_fm[:, :, r0:r0 + pr],
                                       in_=tp[:, :, :pr])

                    mlp_g2(w2q_sb, h1q_nt, "e2", e2_epilogue)

                # ---------------- stage G1 ----------------
                mlp_g1(w1T, b1, ln2t_fm, h1_nt, res2, "tagW1", "g1")

                # ---------------- stage G2 ----------------
                w2_sb = res2.tile([P, KM, E], BF16, tag="tagW2", name="w2_sb")
                for k in range(KM):
                    nc.sync.dma_start(out=w2_sb[:, k, :],
                                      in_=w2T[k * P:(k + 1) * P, :])
                with tc.tile_pool(name="gg", bufs=1) as ggp, \
                     tc.tile_pool(name="gout", bufs=3) as gop:
                    b2b = ggp.tile([P, E], F32, name="g_b2")
                    nc.sync.dma_start(out=b2b, in_=_bcast_ap(b2, P))

                    def g2_epilogue(ps, r0, pr):
                        ot = gop.tile([P, E], F32, name="g_out")
                        nc.vector.tensor_add(ot[:pr], ps[:pr, :], b2b[:pr])
                        nc.sync.dma_start(out=out[r0:r0 + pr, :],
                                          in_=ot[:pr])

                    mlp_g2(w2_sb, h1_nt, "g2", g2_epilogue)

    nc.compile()
    return nc


_NC = None


def _get_nc():
    global _NC
    if _NC is None:
        _NC = build_nc()
    return _NC


def _prep_in_maps(inputs):
    f32 = lambda a: np.ascontiguousarray(np.asarray(a, dtype=np.float32))
    bf = lambda a: np.ascontiguousarray(
        np.asarray(a, dtype=np.float32).astype(ml_dtypes.bfloat16))
    x = f32(inputs["inputs"])                       # (B,HW,NF,E)
    ipw = f32(inputs["in_proj_w"])
    ipb = f32(inputs["in_proj_b"])
    wq, wk, wv = ipw[:E], ipw[E:2 * E], ipw[2 * E:]
    bq, bk_, bv_ = ipb[:E], ipb[E:2 * E], ipb[2 * E:]
    s = 1.0 / np.sqrt(np.float32(D))
    shared = {
        "wqT": bf(wq.T * s), "wkT": bf(wk.T), "wvT": bf(wv.T),
        "woT": bf(f32(inputs["out_proj_w"]).T),
        "w1qT": bf(f32(inputs["mlpq_w1"]).T),
        "w2qT": bf(f32(inputs["mlpq_w2"]).T),
        "w1T": bf(f32(inputs["mlp_w1"]).T),
        "w2T": bf(f32(inputs["mlp_w2"]).T),
        "bqs": f32(bq * s), "bk": f32(bk_), "bv": f32(bv_),
        "bo": f32(inputs["out_proj_b"]),
        "b1q": f32(inputs["mlpq_b1"]), "b2q": f32(inputs["mlpq_b2"]),
        "b1": f32(inputs["mlp_b1"]), "b2": f32(inputs["mlp_b2"]),
        "gq": f32(inputs["ln_q_g"]), "bbq": f32(inputs["ln_q_b"]),
        "gkv": f32(inputs["ln_kv_g"]), "bbkv": f32(inputs["ln_kv_b"]),
        "gres": f32(inputs["res_ln_g"]), "bbres": f32(inputs["res_ln_b"]),
        "gln2": f32(inputs["ln2_g"]), "bbln2": f32(inputs["ln2_b"]),
    }
    return [dict(shared, x=np.ascontiguousarray(x[b].reshape(NTOK, E)))
            for b in range(B)]


def _run(inputs, trace=False):
    from concourse.bass_utils import run_bass_kernel_spmd
    nc = _get_nc()
    in_maps = _prep_in_maps(inputs)
    res = run_bass_kernel_spmd(nc, in_maps, core_ids=list(range(B)),
                               trace=trace)
    outs = np.stack([r["out"].reshape(T, NF, E) for r in res.results])
    return outs, res


def kernel(**inputs) -> np.ndarray:
    outs, _ = _run(inputs, trace=False)
    return outs


# revision 3
# speedup vs baseline: 3.8406x; 3.7083x over previous
"""Trainium2 Bass kernel for LocalBackwardTemporalAttention (optimized v2).

Data-parallel over batch B=8 (one element per core, no collectives).
Single fused on-chip pipeline; SBUF-resident intermediates with
tag-chained slot reuse (two sequential resident pools res1/res2):

  A: LN(x) -> kv_fm (SBUF fm bf16), kv_tm -> DRAM; q_ln -> SBUF
  B: QKV GEMMs from kv_fm -> k_fm, v_res (per-t token-major), q_fm
  C: attention per (hp, t-pair, pi): S = q@k^T (no max-sub) -> exp ->
     sum/recip/scale -> PE-transpose -> attnT@v -> o_fm (SBUF)
  D: out_proj o_fm -> att_fm (SBUF)
  E1: mlpq GEMM1 (+gelu), M-halved weights -> h1q (DRAM)
  --- res1 -> res2 ---
  E2+F: per 128-token chunk: GEMM2 swapped + b2q + kv_tm residual
        -> res_ln -> ln2 -> PE-transpose -> ln2t_fm (SBUF)
  G1: mlp GEMM1 (+gelu), M-halved -> h1 (DRAM)
  G2: GEMM2 swapped + b2 -> out (DRAM, token-major f32)

All GEMMs bf16 in / f32 PSUM accum. DRAM round trips only for kv_tm,
h1q, h1. DMA transfers are 0.125-4 MB (vs ~1100 tiny DMAs in v1).
"""

import sys

sys.path.insert(0, "/opt/trn_rl_repo")

import numpy as np
import ml_dtypes

import concourse.bass as bass
import concourse.bacc as bacc_mod
import concourse.mybir as mybir
import concourse.tile as tile
from concourse.masks import make_identity

F32 = mybir.dt.float32
BF16 = mybir.dt.bfloat16
AF = mybir.ActivationFunctionType
ALU = mybir.AluOpType
AX = mybir.AxisListType

B, HW, NF, E, M, H = 8, 16, 196, 1024, 4096, 16
T, D = HW - 1, E // H            # 15, 64
NKV, NQ, NTOK = T * NF, NF, HW * NF
EPS = 1e-6
P = 128
NT = 490                         # fm GEMM token tile (2940 = 6*490)
KE, KM = E // P, M // P          # 8, 32
EH = E // 2                      # 512
NTILES = NKV // NT               # 6
MH = M // 2                      # 2048


def _ceil(a, b):
    return -(-a // b)


def _bcast_ap(handle, n):
    a = handle[:]
    return bass.AP(tensor=a.tensor, offset=a.offset, ap=[[0, n], list(a.ap[0])])


def _col_ap(handle, mo):
    a = handle[:]
    return bass.AP(tensor=a.tensor, offset=a.offset, ap=[[1, P], [P, mo]])


def build_nc():
    nc = bacc_mod.Bacc(None, target_bir_lowering=False)
    t = lambda n, s, d: nc.dram_tensor(n, s, d, kind="ExternalInput")

    x = t("x", [NTOK, E], F32)
    wqT = t("wqT", [E, E], BF16)
    wkT = t("wkT", [E, E], BF16)
    wvT = t("wvT", [E, E], BF16)
    woT = t("woT", [E, E], BF16)
    w1qT = t("w1qT", [E, M], BF16)
    w2qT = t("w2qT", [M, E], BF16)
    w1T = t("w1T", [E, M], BF16)
    w2T = t("w2T", [M, E], BF16)
    bqs = t("bqs", [E], F32)
    bk = t("bk", [E], F32)
    bv = t("bv", [E], F32)
    bo = t("bo", [E], F32)
    b1q = t("b1q", [M], F32)
    b2q = t("b2q", [E], F32)
    b1 = t("b1", [M], F32)
    b2 = t("b2", [E], F32)
    gq = t("gq", [E], F32)
    bbq = t("bbq", [E], F32)
    gkv = t("gkv", [E], F32)
    bbkv = t("bbkv", [E], F32)
    gres = t("gres", [E], F32)
    bbres = t("bbres", [E], F32)
    gln2 = t("gln2", [E], F32)
    bbln2 = t("bbln2", [E], F32)
    out = nc.dram_tensor("out", [NKV, E], F32, kind="ExternalOutput")

    with tile.TileContext(nc) as tc:
        with tc.tile_pool(name="dram", bufs=1, space="DRAM") as dram, \
             tc.tile_pool(name="consts", bufs=1) as consts:
            kv_tm = dram.tile([NKV, E], BF16)
            h1q_nt = dram.tile([NTILES, P, KM, NT], BF16)
            h1_nt = dram.tile([NTILES, P, KM, NT], BF16)

            ident = consts.tile([P, P], BF16)
            make_identity(nc, ident)
            epst = consts.tile([P, 1], F32)
            nc.vector.memset(epst, EPS)

            # =================== res1: stages A..E1 ===================
            with tc.tile_pool(name="res1", bufs=1) as res1:
                # tagA: kv_fm -> o_fm          (47 KB/part)
                # tagB: k_fm -> att_fm         (47 KB)
                # tagC: v_res -> w1q halves    (60 KB)
                # tagD: qln_fm -> wo           (16 KB)
                # tagE: q_fm                   (3 KB)
                kv_fm = res1.tile([P, KE, NKV], BF16, tag="tagA", name="kv_fm")
                qln_fm = res1.tile([P, KE, NQ], BF16, tag="tagD",
                                   name="qln_fm")

                # ---------------- stage A: LN + transpose ----------------
                def ln_rows(xrows, gain, bias_, n_rows, fm_dst, tm_dst):
                    with tc.tile_pool(name="lnx", bufs=2) as xp, \
                         tc.tile_pool(name="lnst", bufs=4) as stp, \
                         tc.tile_pool(name="lnps", bufs=2, space="PSUM") as psp, \
                         tc.tile_pool(name="lng", bufs=1) as gp:
                        gt = gp.tile([P, E], BF16, name="ln_g")
                        bt = gp.tile([P, E], BF16, name="ln_b")
                        nc.gpsimd.dma_start(out=gt, in_=_bcast_ap(gain, P))
                        nc.gpsimd.dma_start(out=bt, in_=_bcast_ap(bias_, P))
                        for it in range(_ceil(n_rows, P)):
                            r0 = it * P
                            p = min(P, n_rows - r0)
                            xt = xp.tile([P, E], F32, name="ln_x")
                            nc.sync.dma_start(out=xt[:p],
                                              in_=xrows[r0:r0 + p, :])
                            x3 = xt.rearrange("p (n f) -> p n f", n=2)
                            st = stp.tile([P, 2, 6], F32, name="ln_st")
                            for i in range(2):
                                nc.vector.bn_stats(out=st[:p, i, :],
                                                   in_=x3[:p, i, :])
                            mv = stp.tile([P, 2], F32, name="ln_mv")
                            nc.vector.bn_aggr(out=mv[:p], in_=st[:p])
                            rs = stp.tile([P, 1], F32, name="ln_rs")
                            nc.scalar.activation(out=rs[:p], in_=mv[:p, 1:2],
                                                 func=AF.Sqrt, bias=epst[:p])
                            nc.vector.reciprocal(out=rs[:p], in_=rs[:p])
                            y = xp.tile([P, E], BF16, name="ln_y")
                            nc.vector.tensor_scalar(
                                out=y[:p], in0=xt[:p], scalar1=mv[:p, 0:1],
                                scalar2=rs[:p], op0=ALU.subtract,
                                op1=ALU.mult)
                            nc.vector.tensor_mul(y[:p], y[:p], gt[:p])
                            nc.vector.tensor_add(y[:p], y[:p], bt[:p])
                            if tm_dst is not None:
                                nc.sync.dma_start(out=tm_dst[r0:r0 + p, :],
                                                  in_=y[:p])
                            tp = psp.tile([P, KE, P], BF16, name="ln_tp")
                            for e in range(KE):
                                nc.tensor.transpose(
                                    out=tp[:, e, :p],
                                    in_=y[:p, e * P:(e + 1) * P],
                                    identity=ident[:p, :p])
                            nc.scalar.copy(out=fm_dst[:, :, r0:r0 + p],
                                           in_=tp[:, :, :p])

                ln_rows(x[:NKV, :], gkv, bbkv, NKV, kv_fm, kv_tm)
                ln_rows(x[NKV:, :], gq, bbq, NQ, qln_fm, None)

                # ---------------- stage B: QKV ----------------
                k_fm = res1.tile([P, KE, NKV], BF16, tag="tagB", name="k_fm")
                v_res = res1.tile([P, T, 2, E], BF16, tag="tagC",
                                  name="v_res")
                q_fm = res1.tile([P, KE, NQ], BF16, tag="tagE", name="q_fm")

                def fm_gemm(wT_h, bias_h, src_fm, n_total, dst_fm, act, mo,
                            kc, tag):
                    with tc.tile_pool(name=tag + "w", bufs=1) as wp, \
                         tc.tile_pool(name=tag + "b", bufs=1) as bp, \
                         tc.tile_pool(name=tag + "ps", bufs=4,
                                      space="PSUM") as pp:
                        wsb = wp.tile([P, kc, mo * P], BF16, name=tag + "_w")
                        for k in range(kc):
                            nc.sync.dma_start(out=wsb[:, k, :],
                                              in_=wT_h[k * P:(k + 1) * P, :])
                        bsb = bp.tile([P, mo], F32, name=tag + "_b")
                        nc.sync.dma_start(out=bsb, in_=_col_ap(bias_h, mo))
                        for n in range(_ceil(n_total, NT)):
                            n0 = n * NT
                            w = min(NT, n_total - n0)
                            for m in range(mo):
                                ps = pp.tile([P, NT], F32, name=tag + "_ps")
                                for k in range(kc):
                                    nc.tensor.matmul(
                                        ps[:, :w],
                                        lhsT=wsb[:, k, m * P:(m + 1) * P],
                                        rhs=src_fm[:, k, n0:n0 + w],
                                        start=(k == 0), stop=(k == kc - 1))
                                nc.scalar.activation(
                                    out=dst_fm[:, m, n0:n0 + w],
                                    in_=ps[:, :w], func=act,
                                    bias=bsb[:, m:m + 1])

                fm_gemm(wkT, bk, kv_fm, NKV, k_fm, AF.Identity, KE, KE, "bk")
                fm_gemm(wqT, bqs, qln_fm, NQ, q_fm, AF.Identity, KE, KE,
                        "bq")

                # v: token-major per (t, half) via swapped orientation
                with tc.tile_pool(name="vw", bufs=1) as vwp, \
                     tc.tile_pool(name="vb", bufs=1) as vbp, \
                     tc.tile_pool(name="vps", bufs=3, space="PSUM") as vpp:
                    wsb = vwp.tile([P, KE, E], BF16, name="v_w")
                    for k in range(KE):
                        nc.sync.dma_start(out=wsb[:, k, :],
                                          in_=wvT[k * P:(k + 1) * P, :])
                    bvb = vbp.tile([P, E], F32, name="v_b")
                    nc.sync.dma_start(out=bvb, in_=_bcast_ap(bv, P))
                    for t_ in range(T):
                        for half in range(2):
                            r0 = t_ * NF + half * P
                            pr = P if half == 0 else NF - P
                            ps = vpp.tile([P, E], F32, name="v_ps")
                            for j in range(2):
                                for k in range(KE):
                                    nc.tensor.matmul(
                                        ps[:pr, j * EH:(j + 1) * EH],
                                        lhsT=kv_fm[:, k, r0:r0 + pr],
                                        rhs=wsb[:, k, j * EH:(j + 1) * EH],
                                        start=(k == 0), stop=(k == KE - 1))
                            nc.vector.tensor_add(v_res[:pr, t_, half, :],
                                                 ps[:pr, :], bvb[:pr, :])

                # ---------------- stage C: attention ----------------
                o_fm = res1.tile([P, KE, NKV], BF16, tag="tagA", name="o_fm")

                nch = [(0, P), (P, NQ - P)]
                tpairs = [(t0, min(2, T - t0)) for t0 in range(0, T, 2)]
                with tc.tile_pool(name="cat", bufs=3) as cat, \
                     tc.tile_pool(name="cst", bufs=4) as cst, \
                     tc.tile_pool(name="cps", bufs=2, space="PSUM") as cps, \
                     tc.tile_pool(name="cpt", bufs=2, space="PSUM") as cpt, \
                     tc.tile_pool(name="cpo", bufs=2, space="PSUM") as cpo:
                    for hp in range(H // 2):
                        for (tb, tn) in tpairs:
                            po = cpo.tile([P, 2, NQ], F32, name="c_po")
                            for pi in range(2):
                                d0 = pi * D
                                asbs = []
                                rc = cst.tile([P, 2, 2], F32, name="c_rc")
                                sm = cst.tile([P, 2, 2], F32, name="c_sm")
                                for j, (n0, pn) in enumerate(nch):
                                    ps = cps.tile([P, 2 * NQ], F32,
                                                  name=f"c_ps{j}")
                                    nc.tensor.matmul(
                                        ps[:pn, :tn * NQ],
                                        lhsT=q_fm[d0:d0 + D, hp, n0:n0 + pn],
                                        rhs=k_fm[d0:d0 + D, hp,
                                                 tb * NF:(tb + tn) * NF],
                                        start=True, stop=True)
                                    asb = cat.tile([P, 2, NQ], BF16,
                                                   name=f"c_asb{j}")
                                    nc.scalar.activation(
                                        out=asb[:pn, :tn, :].rearrange(
                                            "p t n -> p (t n)"),
                                        in_=ps[:pn, :tn * NQ], func=AF.Exp)
                                    nc.vector.reduce_sum(
                                        out=sm[:pn, j, :tn],
                                        in_=asb[:pn, :tn, :], axis=AX.X)
                                    asbs.append(asb)
                                nc.vector.reciprocal(out=rc, in_=sm)
                                for j, (n0, pn) in enumerate(nch):
                                    for ti in range(tn):
                                        nc.vector.tensor_scalar_mul(
                                            asbs[j][:pn, ti, :],
                                            in0=asbs[j][:pn, ti, :],
                                            scalar1=rc[:pn, j, ti:ti + 1])
                                pt = cpt.tile([P, 2, 2, NQ], BF16,
                                              name="c_pt")
                                for ti in range(tn):
                                    for jn, (n0, pn) in enumerate(nch):
                                        for jm, (m0, mj) in enumerate(nch):
                                            nc.tensor.transpose(
                                                out=pt[:mj, ti, jm,
                                                       n0:n0 + pn],
                                                in_=asbs[jn][:pn, ti,
                                                             m0:m0 + mj],
                                                identity=ident[:pn, :pn])
                                atT = cat.tile([P, 2, 2, NQ], BF16,
                                               name="c_atT")
                                nc.scalar.copy(out=atT[:, :tn],
                                               in_=pt[:, :tn])
                                for ti in range(tn):
                                    for jm, (m0, mj) in enumerate(nch):
                                        nc.tensor.matmul(
                                            po[d0:d0 + D, ti, :],
                                            lhsT=v_res[:mj, tb + ti, jm,
                                                       hp * P + d0:
                                                       hp * P + d0 + D],
                                            rhs=atT[:mj, ti, jm, :],
                                            start=(jm == 0), stop=(jm == 1))
                            nc.scalar.copy(
                                out=o_fm[:, hp, tb * NF:(tb + tn) * NF],
                                in_=po[:, :tn, :])

                # ---------------- stage D: out_proj ----------------
                att_fm = res1.tile([P, KE, NKV], BF16, tag="tagB",
                                   name="att_fm")
                wo_sb = res1.tile([P, KE, E], BF16, tag="tagD", name="wo_sb")
                with tc.tile_pool(name="dob", bufs=1) as dbp, \
                     tc.tile_pool(name="dops", bufs=4, space="PSUM") as dpp:
                    for k in range(KE):
                        nc.sync.dma_start(out=wo_sb[:, k, :],
                                          in_=woT[k * P:(k + 1) * P, :])
                    bsb = dbp.tile([P, KE], F32, name="do_b")
                    nc.sync.dma_start(out=bsb, in_=_col_ap(bo, KE))
                    for n in range(NTILES):
                        n0 = n * NT
                        for m in range(KE):
                            ps = dpp.tile([P, NT], F32, name="do_ps")
                            for k in range(KE):
                                nc.tensor.matmul(
                                    ps[:, :],
                                    lhsT=wo_sb[:, k, m * P:(m + 1) * P],
                                    rhs=o_fm[:, k, n0:n0 + NT],
                                    start=(k == 0), stop=(k == KE - 1))
                            nc.scalar.activation(
                                out=att_fm[:, m, n0:n0 + NT], in_=ps[:, :],
                                func=AF.Identity, bias=bsb[:, m:m + 1])

                # ---------------- stage E1: mlpq GEMM1 ----------------
                def mlp_g1(wT_h, bias_h, src_fm, dst_nt, wpool, wtag,
                           stgpool, stgtag, tag):
                    with tc.tile_pool(name=tag + "b", bufs=1) as bp, \
                         tc.tile_pool(name=tag + "ps", bufs=4,
                                      space="PSUM") as pp:
                        bsb = bp.tile([P, KM], F32, name=tag + "_b")
                        nc.sync.dma_start(out=bsb, in_=_col_ap(bias_h, KM))
                        for mh in range(2):
                            wsb = wpool.tile([P, KE, MH], BF16, tag=wtag,
                                             name=tag + f"_w{mh}")
                            for k in range(KE):
                                nc.sync.dma_start(
                                    out=wsb[:, k, :],
                                    in_=wT_h[k * P:(k + 1) * P,
                                             mh * MH:(mh + 1) * MH])
                            for n in range(NTILES):
                                n0 = n * NT
                                for q in range(2):
                                    stg = stgpool.tile([P, 8, NT], BF16,
                                                       tag=stgtag, bufs=2,
                                                       name=tag + "_stg")
                                    for m8 in range(8):
                                        mi = mh * 16 + q * 8 + m8
                                        ps = pp.tile([P, NT], F32,
                                                     name=tag + "_ps")
                                        for k in range(KE):
                                            nc.tensor.matmul(
                                                ps[:, :],
                                                lhsT=wsb[:, k,
                                                         (q * 8 + m8) * P:
                                                         (q * 8 + m8 + 1) * P],
                                                rhs=src_fm[:, k,
                                                           n0:n0 + NT],
                                                start=(k == 0),
                                                stop=(k == KE - 1))
                                        nc.scalar.activation(
                                            out=stg[:, m8, :], in_=ps[:, :],
                                            func=AF.Gelu,
                                            bias=bsb[:, mi:mi + 1])
                                    nc.sync.dma_start(
                                        out=dst_nt[n][:, mh * 16 + q * 8:
                                                      mh * 16 + q * 8 + 8,
                                                      :],
                                        in_=stg)

                mlp_g1(w1qT, b1q, att_fm, h1q_nt, res1, "tagC", res1, "tagE1S", "e1")

            # =================== res2: stages E2..G2 ===================
            with tc.tile_pool(name="res2", bufs=1) as res2:
                # tagH: ln2t_fm (47 KB); tagW2: w2q -> w2 (64 KB)
                # tagW1: w1 halves (32 KB); quarters via scoped pools
                ln2t_fm = res2.tile([P, KE, NKV], BF16, tag="tagH",
                                    name="ln2t_fm")

                def ln_tm(stp, src, dst, gg, bb, p):
                    x3 = src.rearrange("p (n f) -> p n f", n=2)
                    st = stp.tile([P, 2, 6], F32, name="f_st")
                    for i in range(2):
                        nc.vector.bn_stats(out=st[:p, i, :], in_=x3[:p, i, :])
                    mv = stp.tile([P, 2], F32, name="f_mv")
                    nc.vector.bn_aggr(out=mv[:p], in_=st[:p])
                    rs = stp.tile([P, 1], F32, name="f_rs")
                    nc.scalar.activation(out=rs[:p], in_=mv[:p, 1:2],
                                         func=AF.Sqrt, bias=epst[:p])
                    nc.vector.reciprocal(out=rs[:p], in_=rs[:p])
                    nc.vector.tensor_scalar(
                        out=dst[:p], in0=src[:p], scalar1=mv[:p, 0:1],
                        scalar2=rs[:p], op0=ALU.subtract, op1=ALU.mult)
                    nc.vector.tensor_mul(dst[:p], dst[:p], gg[:p])
                    nc.vector.tensor_add(dst[:p], dst[:p], bb[:p])

                def mlp_g2(w_sb, src_nt, tag, epilogue):
                    with tc.tile_pool(name=tag + "ps", bufs=2,
                                      space="PSUM") as pp:
                        for n in range(NTILES):
                            hq = res2.tile([P, KM, NT], BF16, tag="tagW1",
                                           name=tag + "_h")
                            for qtr in range(4):
                                nc.sync.dma_start(
                                    out=hq[:, qtr * 8:(qtr + 1) * 8, :],
                                    in_=src_nt[n][:, qtr * 8:(qtr + 1) * 8,
                                                  :])
                            subs = [(0, P), (P, P), (2 * P, P),
                                    (3 * P, NT - 3 * P)]
                            for (s0, pr) in subs:
                                ps = pp.tile([P, E], F32, name=tag + "_ps")
                                for j in range(2):
                                    for k in range(KM):
                                        nc.tensor.matmul(
                                            ps[:pr, j * EH:(j + 1) * EH],
                                            lhsT=hq[:, k, s0:s0 + pr],
                                            rhs=w_sb[:, k,
                                                     j * EH:(j + 1) * EH],
                                            start=(k == 0),
                                            stop=(k == KM - 1))
                                epilogue(ps, n * NT + s0, pr)

                # ---------------- stage E2 + F ----------------
                w2q_sb = res2.tile([P, KM, E], BF16, tag="tagW2",
                                   name="w2q_sb")
                for k in range(KM):
                    nc.sync.dma_start(out=w2q_sb[:, k, :],
                                      in_=w2qT[k * P:(k + 1) * P, :])
                with tc.tile_pool(name="fg", bufs=1) as fg, \
                     tc.tile_pool(name="fst", bufs=4) as fst, \
                     tc.tile_pool(name="fw", bufs=2) as fwp, \
                     tc.tile_pool(name="fkv", bufs=2) as fkv, \
                     tc.tile_pool(name="ftp", bufs=2, space="PSUM") as ftp:
                    b2qb = fg.tile([P, E], F32, name="f_b2q")
                    g1t = fg.tile([P, E], BF16, name="f_g1")
                    b1t = fg.tile([P, E], BF16, name="f_b1")
                    g2t = fg.tile([P, E], BF16, name="f_g2")
                    b2t = fg.tile([P, E], BF16, name="f_b2")
                    nc.sync.dma_start(out=b2qb, in_=_bcast_ap(b2q, P))
                    nc.gpsimd.dma_start(out=g1t, in_=_bcast_ap(gres, P))
                    nc.gpsimd.dma_start(out=b1t, in_=_bcast_ap(bbres, P))
                    nc.gpsimd.dma_start(out=g2t, in_=_bcast_ap(gln2, P))
                    nc.gpsimd.dma_start(out=b2t, in_=_bcast_ap(bbln2, P))

                    def e2_epilogue(ps, r0, pr):
                        kvt = fkv.tile([P, E], BF16, name="f_kv")
                        nc.sync.dma_start(out=kvt[:pr],
                                          in_=kv_tm[r0:r0 + pr, :])
                        qs = fwp.tile([P, E], F32, name="f_qs")
                        nc.vector.tensor_add(qs[:pr], ps[:pr, :], b2qb[:pr])
                        nc.vector.tensor_add(qs[:pr], qs[:pr], kvt[:pr])
                        y1 = fwp.tile([P, E], F32, name="f_y1")
                        ln_tm(fst, qs, y1, g1t, b1t, pr)
                        y2 = fwp.tile([P, E], BF16, name="f_y2")
                        ln_tm(fst, y1, y2, g2t, b2t, pr)
                        tp = ftp.tile([P, KE, P], BF16, name="f_tp")
                        for e in range(KE):
                            nc.tensor.transpose(
                                out=tp[:, e, :pr],
                                in_=y2[:pr, e * P:(e + 1) * P],
                                identity=ident[:pr, :pr])
                        nc.scalar.copy(out=ln2t_fm[:, :, r0:r0 + pr],
                                       in_=tp[:, :, :pr])

                    mlp_g2(w2q_sb, h1q_nt, "e2", e2_epilogue)

                # ---------------- stage G1 ----------------
                mlp_g1(w1T, b1, ln2t_fm, h1_nt, res2, "tagW1", res2, "tagG1S", "g1")

                # ---------------- stage G2 ----------------
                w2_sb = res2.tile([P, KM, E], BF16, tag="tagW2", name="w2_sb")
                for k in range(KM):
                    nc.sync.dma_start(out=w2_sb[:, k, :],
                                      in_=w2T[k * P:(k + 1) * P, :])
                with tc.tile_pool(name="gg", bufs=1) as ggp, \
                     tc.tile_pool(name="gout", bufs=3) as gop:
                    b2b = ggp.tile([P, E], F32, name="g_b2")
                    nc.sync.dma_start(out=b2b, in_=_bcast_ap(b2, P))

                    def g2_epilogue(ps, r0, pr):
                        ot = gop.tile([P, E], F32, name="g_out")
                        nc.vector.tensor_add(ot[:pr], ps[:pr, :], b2b[:pr])
                        nc.sync.dma_start(out=out[r0:r0 + pr, :],
                                          in_=ot[:pr])

                    mlp_g2(w2_sb, h1_nt, "g2", g2_epilogue)

    nc.compile()
    return nc


_NC = None


def _get_nc():
    global _NC
    if _NC is None:
        _NC = build_nc()
    return _NC


def _prep_in_maps(inputs):
    f32 = lambda a: np.ascontiguousarray(np.asarray(a, dtype=np.float32))
    bf = lambda a: np.ascontiguousarray(
        np.asarray(a, dtype=np.float32).astype(ml_dtypes.bfloat16))
    x = f32(inputs["inputs"])                       # (B,HW,NF,E)
    ipw = f32(inputs["in_proj_w"])
    ipb = f32(inputs["in_proj_b"])
    wq, wk, wv = ipw[:E], ipw[E:2 * E], ipw[2 * E:]
    bq, bk_, bv_ = ipb[:E], ipb[E:2 * E], ipb[2 * E:]
    s = 1.0 / np.sqrt(np.float32(D))
    shared = {
        "wqT": bf(wq.T * s), "wkT": bf(wk.T), "wvT": bf(wv.T),
        "woT": bf(f32(inputs["out_proj_w"]).T),
        "w1qT": bf(f32(inputs["mlpq_w1"]).T),
        "w2qT": bf(f32(inputs["mlpq_w2"]).T),
        "w1T": bf(f32(inputs["mlp_w1"]).T),
        "w2T": bf(f32(inputs["mlp_w2"]).T),
        "bqs": f32(bq * s), "bk": f32(bk_), "bv": f32(bv_),
        "bo": f32(inputs["out_proj_b"]),
        "b1q": f32(inputs["mlpq_b1"]), "b2q": f32(inputs["mlpq_b2"]),
        "b1": f32(inputs["mlp_b1"]), "b2": f32(inputs["mlp_b2"]),
        "gq": f32(inputs["ln_q_g"]), "bbq": f32(inputs["ln_q_b"]),
        "gkv": f32(inputs["ln_kv_g"]), "bbkv": f32(inputs["ln_kv_b"]),
        "gres": f32(inputs["res_ln_g"]), "bbres": f32(inputs["res_ln_b"]),
        "gln2": f32(inputs["ln2_g"]), "bbln2": f32(inputs["ln2_b"]),
    }
    return [dict(shared, x=np.ascontiguousarray(x[b].reshape(NTOK, E)))
            for b in range(B)]


def _run(inputs, trace=False):
    from concourse.bass_utils import run_bass_kernel_spmd
    nc = _get_nc()
    in_maps = _prep_in_maps(inputs)
    res = run_bass_kernel_spmd(nc, in_maps, core_ids=list(range(B)),
                               trace=trace)
    outs = np.stack([r["out"].reshape(T, NF, E) for r in res.results])
    return outs, res


def kernel(**inputs) -> np.ndarray:
    outs, _ = _run(inputs, trace=False)
    return outs


# revision 4
# speedup vs baseline: 4.2793x; 1.1142x over previous
"""Trainium2 Bass kernel for LocalBackwardTemporalAttention (optimized v2).

Data-parallel over batch B=8 (one element per core, no collectives).
Single fused on-chip pipeline; SBUF-resident intermediates with
tag-chained slot reuse (two sequential resident pools res1/res2):

  A: LN(x) -> kv_fm (SBUF fm bf16), kv_tm -> DRAM; q_ln -> SBUF
  B: QKV GEMMs from kv_fm -> k_fm, v_res (per-t token-major), q_fm
  C: attention per (hp, t-pair, pi): S = q@k^T (no max-sub) -> exp ->
     sum/recip/scale -> PE-transpose -> attnT@v -> o_fm (SBUF)
  D: out_proj o_fm -> att_fm (SBUF)
  E1: mlpq GEMM1 (+gelu), M-halved weights -> h1q (DRAM)
  --- res1 -> res2 ---
  E2+F: per 128-token chunk: GEMM2 swapped + b2q + kv_tm residual
        -> res_ln -> ln2 -> PE-transpose -> ln2t_fm (SBUF)
  G1: mlp GEMM1 (+gelu), M-halved -> h1 (DRAM)
  G2: GEMM2 swapped + b2 -> out (DRAM, token-major f32)

All GEMMs bf16 in / f32 PSUM accum. DRAM round trips only for kv_tm,
h1q, h1. DMA transfers are 0.125-4 MB (vs ~1100 tiny DMAs in v1).
"""

import sys

sys.path.insert(0, "/opt/trn_rl_repo")

import numpy as np
import ml_dtypes

import concourse.bass as bass
import concourse.bacc as bacc_mod
import concourse.mybir as mybir
import concourse.tile as tile
from concourse.masks import make_identity

F32 = mybir.dt.float32
BF16 = mybir.dt.bfloat16
AF = mybir.ActivationFunctionType
ALU = mybir.AluOpType
AX = mybir.AxisListType

B, HW, NF, E, M, H = 8, 16, 196, 1024, 4096, 16
T, D = HW - 1, E // H            # 15, 64
NKV, NQ, NTOK = T * NF, NF, HW * NF
EPS = 1e-6
P = 128
NT = 490                         # fm GEMM token tile (2940 = 6*490)
KE, KM = E // P, M // P          # 8, 32
EH = E // 2                      # 512
NTILES = NKV // NT               # 6
MH = M // 2                      # 2048


def _ceil(a, b):
    return -(-a // b)


def _bcast_ap(handle, n):
    a = handle[:]
    return bass.AP(tensor=a.tensor, offset=a.offset, ap=[[0, n], list(a.ap[0])])


def _col_ap(handle, mo):
    a = handle[:]
    return bass.AP(tensor=a.tensor, offset=a.offset, ap=[[1, P], [P, mo]])


def build_nc():
    nc = bacc_mod.Bacc(None, target_bir_lowering=False)
    t = lambda n, s, d: nc.dram_tensor(n, s, d, kind="ExternalInput")

    x = t("x", [NTOK, E], F32)
    wqT = t("wqT", [E, E], BF16)
    wkT = t("wkT", [E, E], BF16)
    wvT = t("wvT", [E, E], BF16)
    woT = t("woT", [E, E], BF16)
    w1qT = t("w1qT", [E, M], BF16)
    w2qT = t("w2qT", [M, E], BF16)
    w1T = t("w1T", [E, M], BF16)
    w2T = t("w2T", [M, E], BF16)
    bqs = t("bqs", [E], F32)
    bk = t("bk", [E], F32)
    bv = t("bv", [E], F32)
    bo = t("bo", [E], F32)
    b1q = t("b1q", [M], F32)
    b2q = t("b2q", [E], F32)
    b1 = t("b1", [M], F32)
    b2 = t("b2", [E], F32)
    gq = t("gq", [E], F32)
    bbq = t("bbq", [E], F32)
    gkv = t("gkv", [E], F32)
    bbkv = t("bbkv", [E], F32)
    gres = t("gres", [E], F32)
    bbres = t("bbres", [E], F32)
    gln2 = t("gln2", [E], F32)
    bbln2 = t("bbln2", [E], F32)
    out = nc.dram_tensor("out", [NKV, E], F32, kind="ExternalOutput")

    with tile.TileContext(nc) as tc:
        with tc.tile_pool(name="dram", bufs=1, space="DRAM") as dram, \
             tc.tile_pool(name="consts", bufs=1) as consts:
            kv_tm = dram.tile([NKV, E], BF16)
            h1q_nt = dram.tile([NTILES, P, KM, NT], BF16)
            h1_nt = dram.tile([NTILES, P, KM, NT], BF16)

            ident = consts.tile([P, P], BF16)
            make_identity(nc, ident)
            epst = consts.tile([P, 1], F32)
            nc.vector.memset(epst, EPS)

            # =================== res1: stages A..E1 ===================
            with tc.tile_pool(name="res1", bufs=1) as res1:
                # tagA: kv_fm -> o_fm          (47 KB/part)
                # tagB: k_fm -> att_fm         (47 KB)
                # tagC: v_res -> w1q halves    (60 KB)
                # tagD: qln_fm -> wo           (16 KB)
                # tagE: q_fm                   (3 KB)
                kv_fm = res1.tile([P, KE, NKV], BF16, tag="tagA", name="kv_fm")
                qln_fm = res1.tile([P, KE, NQ], BF16, tag="tagD",
                                   name="qln_fm")

                # ---------------- stage A: LN + transpose ----------------
                def ln_rows(xrows, gain, bias_, n_rows, fm_dst, tm_dst):
                    with tc.tile_pool(name="lnx", bufs=2) as xp, \
                         tc.tile_pool(name="lnst", bufs=4) as stp, \
                         tc.tile_pool(name="lnps", bufs=2, space="PSUM") as psp, \
                         tc.tile_pool(name="lng", bufs=1) as gp:
                        gt = gp.tile([P, E], BF16, name="ln_g")
                        bt = gp.tile([P, E], BF16, name="ln_b")
                        nc.gpsimd.dma_start(out=gt, in_=_bcast_ap(gain, P))
                        nc.gpsimd.dma_start(out=bt, in_=_bcast_ap(bias_, P))
                        for it in range(_ceil(n_rows, P)):
                            r0 = it * P
                            p = min(P, n_rows - r0)
                            xt = xp.tile([P, E], F32, name="ln_x")
                            nc.sync.dma_start(out=xt[:p],
                                              in_=xrows[r0:r0 + p, :])
                            x3 = xt.rearrange("p (n f) -> p n f", n=2)
                            st = stp.tile([P, 2, 6], F32, name="ln_st")
                            for i in range(2):
                                nc.vector.bn_stats(out=st[:p, i, :],
                                                   in_=x3[:p, i, :])
                            mv = stp.tile([P, 2], F32, name="ln_mv")
                            nc.vector.bn_aggr(out=mv[:p], in_=st[:p])
                            rs = stp.tile([P, 1], F32, name="ln_rs")
                            nc.scalar.activation(out=rs[:p], in_=mv[:p, 1:2],
                                                 func=AF.Sqrt, bias=epst[:p])
                            nc.vector.reciprocal(out=rs[:p], in_=rs[:p])
                            y = xp.tile([P, E], BF16, name="ln_y")
                            nc.vector.tensor_scalar(
                                out=y[:p], in0=xt[:p], scalar1=mv[:p, 0:1],
                                scalar2=rs[:p], op0=ALU.subtract,
                                op1=ALU.mult)
                            nc.vector.tensor_mul(y[:p], y[:p], gt[:p])
                            nc.vector.tensor_add(y[:p], y[:p], bt[:p])
                            if tm_dst is not None:
                                nc.sync.dma_start(out=tm_dst[r0:r0 + p, :],
                                                  in_=y[:p])
                            tp = psp.tile([P, KE, P], BF16, name="ln_tp")
                            for e in range(KE):
                                nc.tensor.transpose(
                                    out=tp[:, e, :p],
                                    in_=y[:p, e * P:(e + 1) * P],
                                    identity=ident[:p, :p])
                            nc.scalar.copy(out=fm_dst[:, :, r0:r0 + p],
                                           in_=tp[:, :, :p])

                ln_rows(x[:NKV, :], gkv, bbkv, NKV, kv_fm, kv_tm)
                ln_rows(x[NKV:, :], gq, bbq, NQ, qln_fm, None)

                # ---------------- stage B: QKV ----------------
                k_fm = res1.tile([P, KE, NKV], BF16, tag="tagB", name="k_fm")
                v_res = res1.tile([P, T, 2, E], BF16, tag="tagC",
                                  name="v_res")
                q_fm = res1.tile([P, KE, NQ], BF16, tag="tagE", name="q_fm")

                def fm_gemm(w_src, bias_h, src_fm, n_total, dst_fm, act,
                            mo, kc, tag):
                    with tc.tile_pool(name=tag + "w", bufs=1) as wp, \
                         tc.tile_pool(name=tag + "b", bufs=1) as bp, \
                         tc.tile_pool(name=tag + "ps", bufs=4,
                                      space="PSUM") as pp:
                        if not isinstance(w_src, bass.DRamTensorHandle):
                            wsb = w_src
                        else:
                            wsb = wp.tile([P, kc, mo * P], BF16,
                                          name=tag + "_w")
                            for k in range(kc):
                                nc.sync.dma_start(
                                    out=wsb[:, k, :],
                                    in_=w_src[k * P:(k + 1) * P, :])
                        bsb = bp.tile([P, mo], F32, name=tag + "_b")
                        nc.sync.dma_start(out=bsb, in_=_col_ap(bias_h, mo))
                        for n in range(_ceil(n_total, NT)):
                            n0 = n * NT
                            w = min(NT, n_total - n0)
                            for m in range(mo):
                                ps = pp.tile([P, NT], F32, name=tag + "_ps")
                                for k in range(kc):
                                    nc.tensor.matmul(
                                        ps[:, :w],
                                        lhsT=wsb[:, k, m * P:(m + 1) * P],
                                        rhs=src_fm[:, k, n0:n0 + w],
                                        start=(k == 0), stop=(k == kc - 1))
                                nc.scalar.activation(
                                    out=dst_fm[:, m, n0:n0 + w],
                                    in_=ps[:, :w], func=act,
                                    bias=bsb[:, m:m + 1])

                fm_gemm(wkT, bk, kv_fm, NKV, k_fm, AF.Identity, KE, KE, "bk")
                fm_gemm(wqT, bqs, qln_fm, NQ, q_fm, AF.Identity, KE, KE,
                        "bq")

                # v: token-major per (t, half) via swapped orientation
                with tc.tile_pool(name="vw", bufs=1) as vwp, \
                     tc.tile_pool(name="vb", bufs=1) as vbp, \
                     tc.tile_pool(name="vps", bufs=3, space="PSUM") as vpp:
                    wsb = vwp.tile([P, KE, E], BF16, name="v_w")
                    for k in range(KE):
                        nc.sync.dma_start(out=wsb[:, k, :],
                                          in_=wvT[k * P:(k + 1) * P, :])
                    bvb = vbp.tile([P, E], F32, name="v_b")
                    nc.sync.dma_start(out=bvb, in_=_bcast_ap(bv, P))
                    for t_ in range(T):
                        for half in range(2):
                            r0 = t_ * NF + half * P
                            pr = P if half == 0 else NF - P
                            ps = vpp.tile([P, E], F32, name="v_ps")
                            for j in range(2):
                                for k in range(KE):
                                    nc.tensor.matmul(
                                        ps[:pr, j * EH:(j + 1) * EH],
                                        lhsT=kv_fm[:, k, r0:r0 + pr],
                                        rhs=wsb[:, k, j * EH:(j + 1) * EH],
                                        start=(k == 0), stop=(k == KE - 1))
                            nc.vector.tensor_add(v_res[:pr, t_, half, :],
                                                 ps[:pr, :], bvb[:pr, :])

                # ---------------- stage C: attention ----------------
                o_fm = res1.tile([P, KE, NKV], BF16, tag="tagA", name="o_fm")

                nch = [(0, P), (P, NQ - P)]
                tpairs = [(t0, min(2, T - t0)) for t0 in range(0, T, 2)]
                with tc.tile_pool(name="cat", bufs=3) as cat, \
                     tc.tile_pool(name="cst", bufs=4) as cst, \
                     tc.tile_pool(name="cps", bufs=2, space="PSUM") as cps, \
                     tc.tile_pool(name="cpt", bufs=2, space="PSUM") as cpt, \
                     tc.tile_pool(name="cpo", bufs=2, space="PSUM") as cpo:
                    for hp in range(H // 2):
                        for (tb, tn) in tpairs:
                            po = cpo.tile([P, 2, NQ], F32, name="c_po")
                            for pi in range(2):
                                d0 = pi * D
                                asbs = []
                                rc = cst.tile([P, 2, 2], F32, name="c_rc")
                                sm = cst.tile([P, 2, 2], F32, name="c_sm")
                                for j, (n0, pn) in enumerate(nch):
                                    ps = cps.tile([P, 2 * NQ], F32,
                                                  name=f"c_ps{j}")
                                    nc.tensor.matmul(
                                        ps[:pn, :tn * NQ],
                                        lhsT=q_fm[d0:d0 + D, hp, n0:n0 + pn],
                                        rhs=k_fm[d0:d0 + D, hp,
                                                 tb * NF:(tb + tn) * NF],
                                        start=True, stop=True)
                                    asb = cat.tile([P, 2, NQ], BF16,
                                                   name=f"c_asb{j}")
                                    nc.scalar.activation(
                                        out=asb[:pn, :tn, :].rearrange(
                                            "p t n -> p (t n)"),
                                        in_=ps[:pn, :tn * NQ], func=AF.Exp)
                                    nc.vector.reduce_sum(
                                        out=sm[:pn, j, :tn],
                                        in_=asb[:pn, :tn, :], axis=AX.X)
                                    asbs.append(asb)
                                nc.vector.reciprocal(out=rc, in_=sm)
                                for j, (n0, pn) in enumerate(nch):
                                    for ti in range(tn):
                                        nc.vector.tensor_scalar_mul(
                                            asbs[j][:pn, ti, :],
                                            in0=asbs[j][:pn, ti, :],
                                            scalar1=rc[:pn, j, ti:ti + 1])
                                pt = cpt.tile([P, 2, 2, NQ], BF16,
                                              name="c_pt")
                                for ti in range(tn):
                                    for jn, (n0, pn) in enumerate(nch):
                                        for jm, (m0, mj) in enumerate(nch):
                                            nc.tensor.transpose(
                                                out=pt[:mj, ti, jm,
                                                       n0:n0 + pn],
                                                in_=asbs[jn][:pn, ti,
                                                             m0:m0 + mj],
                                                identity=ident[:pn, :pn])
                                atT = cat.tile([P, 2, 2, NQ], BF16,
                                               name="c_atT")
                                nc.scalar.copy(out=atT[:, :tn],
                                               in_=pt[:, :tn])
                                for ti in range(tn):
                                    for jm, (m0, mj) in enumerate(nch):
                                        nc.tensor.matmul(
                                            po[d0:d0 + D, ti, :],
                                            lhsT=v_res[:mj, tb + ti, jm,
                                                       hp * P + d0:
                                                       hp * P + d0 + D],
                                            rhs=atT[:mj, ti, jm, :],
                                            start=(jm == 0), stop=(jm == 1))
                            nc.scalar.copy(
                                out=o_fm[:, hp, tb * NF:(tb + tn) * NF],
                                in_=po[:, :tn, :])

                # ---------------- stage D: out_proj ----------------
                att_fm = res1.tile([P, KE, NKV], BF16, tag="tagB",
                                   name="att_fm")
                wo_sb = res1.tile([P, KE, E], BF16, tag="tagD", name="wo_sb")
                with tc.tile_pool(name="dob", bufs=1) as dbp, \
                     tc.tile_pool(name="dops", bufs=4, space="PSUM") as dpp:
                    for k in range(KE):
                        nc.sync.dma_start(out=wo_sb[:, k, :],
                                          in_=woT[k * P:(k + 1) * P, :])
                    bsb = dbp.tile([P, KE], F32, name="do_b")
                    nc.sync.dma_start(out=bsb, in_=_col_ap(bo, KE))
                    for n in range(NTILES):
                        n0 = n * NT
                        for m in range(KE):
                            ps = dpp.tile([P, NT], F32, name="do_ps")
                            for k in range(KE):
                                nc.tensor.matmul(
                                    ps[:, :],
                                    lhsT=wo_sb[:, k, m * P:(m + 1) * P],
                                    rhs=o_fm[:, k, n0:n0 + NT],
                                    start=(k == 0), stop=(k == KE - 1))
                            nc.scalar.activation(
                                out=att_fm[:, m, n0:n0 + NT], in_=ps[:, :],
                                func=AF.Identity, bias=bsb[:, m:m + 1])

                # ---------------- stage E1: mlpq GEMM1 ----------------
                def mlp_g1(wT_h, bias_h, src_fm, dst_nt, wpool, wtag,
                           stgpool, stgtag, tag):
                    with tc.tile_pool(name=tag + "b", bufs=1) as bp, \
                         tc.tile_pool(name=tag + "ps", bufs=4,
                                      space="PSUM") as pp:
                        bsb = bp.tile([P, KM], F32, name=tag + "_b")
                        nc.sync.dma_start(out=bsb, in_=_col_ap(bias_h, KM))
                        for mh in range(2):
                            wsb = wpool.tile([P, KE, MH], BF16, tag=wtag,
                                             name=tag + f"_w{mh}")
                            for k in range(KE):
                                nc.sync.dma_start(
                                    out=wsb[:, k, :],
                                    in_=wT_h[k * P:(k + 1) * P,
                                             mh * MH:(mh + 1) * MH])
                            for n in range(NTILES):
                                n0 = n * NT
                                for q in range(2):
                                    stg = stgpool.tile([P, 8, NT], BF16,
                                                       tag=stgtag, bufs=2,
                                                       name=tag + "_stg")
                                    for m8 in range(8):
                                        mi = mh * 16 + q * 8 + m8
                                        ps = pp.tile([P, NT], F32,
                                                     name=tag + "_ps")
                                        for k in range(KE):
                                            nc.tensor.matmul(
                                                ps[:, :],
                                                lhsT=wsb[:, k,
                                                         (q * 8 + m8) * P:
                                                         (q * 8 + m8 + 1) * P],
                                                rhs=src_fm[:, k,
                                                           n0:n0 + NT],
                                                start=(k == 0),
                                                stop=(k == KE - 1))
                                        nc.scalar.activation(
                                            out=stg[:, m8, :], in_=ps[:, :],
                                            func=AF.Gelu,
                                            bias=bsb[:, mi:mi + 1])
                                    nc.sync.dma_start(
                                        out=dst_nt[n][:, mh * 16 + q * 8:
                                                      mh * 16 + q * 8 + 8,
                                                      :],
                                        in_=stg)

                mlp_g1(w1qT, b1q, att_fm, h1q_nt, res1, "tagC", res1, "tagE1S", "e1")

            # =================== res2: stages E2..G2 ===================
            with tc.tile_pool(name="res2", bufs=1) as res2:
                # tagH: ln2t_fm (47 KB); tagW2: w2q -> w2 (64 KB)
                # tagW1: w1 halves (32 KB); quarters via scoped pools
                ln2t_fm = res2.tile([P, KE, NKV], BF16, tag="tagH",
                                    name="ln2t_fm")

                def ln_tm(stp, src, dst, gg, bb, p):
                    x3 = src.rearrange("p (n f) -> p n f", n=2)
                    st = stp.tile([P, 2, 6], F32, name="f_st")
                    for i in range(2):
                        nc.vector.bn_stats(out=st[:p, i, :], in_=x3[:p, i, :])
                    mv = stp.tile([P, 2], F32, name="f_mv")
                    nc.vector.bn_aggr(out=mv[:p], in_=st[:p])
                    rs = stp.tile([P, 1], F32, name="f_rs")
                    nc.scalar.activation(out=rs[:p], in_=mv[:p, 1:2],
                                         func=AF.Sqrt, bias=epst[:p])
                    nc.vector.reciprocal(out=rs[:p], in_=rs[:p])
                    nc.vector.tensor_scalar(
                        out=dst[:p], in0=src[:p], scalar1=mv[:p, 0:1],
                        scalar2=rs[:p], op0=ALU.subtract, op1=ALU.mult)
                    nc.vector.tensor_mul(dst[:p], dst[:p], gg[:p])
                    nc.vector.tensor_add(dst[:p], dst[:p], bb[:p])

                def mlp_g2(w_sb, src_nt, tag, epilogue, hq_first=None):
                    with tc.tile_pool(name=tag + "ps", bufs=2,
                                      space="PSUM") as pp:
                        for n in range(NTILES):
                            if n == 0 and hq_first is not None:
                                hqa, hqb = hq_first
                            else:
                                hqa = res2.tile([P, KM // 2, NT], BF16,
                                                tag="tagW1",
                                                name=tag + "_ha")
                                hqb = res2.tile([P, KM // 2, NT], BF16,
                                                tag="tagW1B",
                                                name=tag + "_hb")
                                for qtr in range(2):
                                    nc.sync.dma_start(
                                        out=hqa[:, qtr * 8:(qtr + 1) * 8,
                                                :],
                                        in_=src_nt[n][:,
                                                      qtr * 8:(qtr + 1) * 8,
                                                      :])
                                    nc.sync.dma_start(
                                        out=hqb[:, qtr * 8:(qtr + 1) * 8,
                                                :],
                                        in_=src_nt[n][:,
                                                      16 + qtr * 8:
                                                      16 + (qtr + 1) * 8,
                                                      :])
                            subs = [(0, P), (P, P), (2 * P, P),
                                    (3 * P, NT - 3 * P)]
                            for (s0, pr) in subs:
                                ps = pp.tile([P, E], F32, name=tag + "_ps")
                                for j in range(2):
                                    for k in range(KM):
                                        hk = hqa if k < 16 else hqb
                                        nc.tensor.matmul(
                                            ps[:pr, j * EH:(j + 1) * EH],
                                            lhsT=hk[:, k % 16, s0:s0 + pr],
                                            rhs=w_sb[:, k,
                                                     j * EH:(j + 1) * EH],
                                            start=(k == 0),
                                            stop=(k == KM - 1))
                                epilogue(ps, n * NT + s0, pr)

                # ---------------- stage E2 + F ----------------
                # load the first GEMM2 input tile before the 8 MB weight so
                # the first matmuls only wait for the leading weight chunks
                hq0a = res2.tile([P, KM // 2, NT], BF16, tag="tagW1",
                                 name="e2_h0a")
                hq0b = res2.tile([P, KM // 2, NT], BF16, tag="tagW1B",
                                 name="e2_h0b")
                for qtr in range(2):
                    nc.sync.dma_start(
                        out=hq0a[:, qtr * 8:(qtr + 1) * 8, :],
                        in_=h1q_nt[0][:, qtr * 8:(qtr + 1) * 8, :])
                    nc.sync.dma_start(
                        out=hq0b[:, qtr * 8:(qtr + 1) * 8, :],
                        in_=h1q_nt[0][:, 16 + qtr * 8:16 + (qtr + 1) * 8, :])
                w2q_sb = res2.tile([P, KM, E], BF16, tag="tagW2",
                                   name="w2q_sb")
                for k in range(KM):
                    nc.sync.dma_start(out=w2q_sb[:, k, :],
                                      in_=w2qT[k * P:(k + 1) * P, :])
                with tc.tile_pool(name="fg", bufs=1) as fg, \
                     tc.tile_pool(name="fst", bufs=4) as fst, \
                     tc.tile_pool(name="fw", bufs=2) as fwp, \
                     tc.tile_pool(name="fkv", bufs=2) as fkv, \
                     tc.tile_pool(name="ftp", bufs=2, space="PSUM") as ftp:
                    b2qb = fg.tile([P, E], BF16, name="f_b2q")
                    g1t = fg.tile([P, E], BF16, name="f_g1")
                    b1t = fg.tile([P, E], BF16, name="f_b1")
                    g2t = fg.tile([P, E], BF16, name="f_g2")
                    b2t = fg.tile([P, E], BF16, name="f_b2")
                    nc.gpsimd.dma_start(out=b2qb, in_=_bcast_ap(b2q, P))
                    nc.gpsimd.dma_start(out=g1t, in_=_bcast_ap(gres, P))
                    nc.gpsimd.dma_start(out=b1t, in_=_bcast_ap(bbres, P))
                    nc.gpsimd.dma_start(out=g2t, in_=_bcast_ap(gln2, P))
                    nc.gpsimd.dma_start(out=b2t, in_=_bcast_ap(bbln2, P))

                    def e2_epilogue(ps, r0, pr):
                        kvt = fkv.tile([P, E], BF16, name="f_kv")
                        nc.sync.dma_start(out=kvt[:pr],
                                          in_=kv_tm[r0:r0 + pr, :])
                        qs = fwp.tile([P, E], F32, name="f_qs")
                        nc.vector.tensor_add(qs[:pr], ps[:pr, :], b2qb[:pr])
                        nc.vector.tensor_add(qs[:pr], qs[:pr], kvt[:pr])
                        y1 = fwp.tile([P, E], F32, name="f_y1")
                        ln_tm(fst, qs, y1, g1t, b1t, pr)
                        y2 = fwp.tile([P, E], BF16, name="f_y2")
                        ln_tm(fst, y1, y2, g2t, b2t, pr)
                        tp = ftp.tile([P, KE, P], BF16, name="f_tp")
                        for e in range(KE):
                            nc.tensor.transpose(
                                out=tp[:, e, :pr],
                                in_=y2[:pr, e * P:(e + 1) * P],
                                identity=ident[:pr, :pr])
                        nc.scalar.copy(out=ln2t_fm[:, :, r0:r0 + pr],
                                       in_=tp[:, :, :pr])

                    mlp_g2(w2q_sb, h1q_nt, "e2", e2_epilogue,
                           hq_first=(hq0a, hq0b))

                # ---------------- stage G1 ----------------
                mlp_g1(w1T, b1, ln2t_fm, h1_nt, res2, "tagW1", res2, "tagG1S", "g1")

                # ---------------- stage G2 ----------------
                w2_sb = res2.tile([P, KM, E], BF16, tag="tagW2", name="w2_sb")
                for k in range(KM):
                    nc.sync.dma_start(out=w2_sb[:, k, :],
                                      in_=w2T[k * P:(k + 1) * P, :])
                with tc.tile_pool(name="gg", bufs=1) as ggp, \
                     tc.tile_pool(name="gout", bufs=2) as gop:
                    b2b = ggp.tile([P, E], F32, name="g_b2")
                    nc.sync.dma_start(out=b2b, in_=_bcast_ap(b2, P))

                    def g2_epilogue(ps, r0, pr):
                        ot = gop.tile([P, E], F32, name="g_out")
                        nc.vector.tensor_add(ot[:pr], ps[:pr, :], b2b[:pr])
                        nc.sync.dma_start(out=out[r0:r0 + pr, :],
                                          in_=ot[:pr])

                    mlp_g2(w2_sb, h1_nt, "g2", g2_epilogue)

    nc.compile()
    return nc


_NC = None


def _get_nc():
    global _NC
    if _NC is None:
        _NC = build_nc()
    return _NC


def _prep_in_maps(inputs):
    f32 = lambda a: np.ascontiguousarray(np.asarray(a, dtype=np.float32))
    bf = lambda a: np.ascontiguousarray(
        np.asarray(a, dtype=np.float32).astype(ml_dtypes.bfloat16))
    x = f32(inputs["inputs"])                       # (B,HW,NF,E)
    ipw = f32(inputs["in_proj_w"])
    ipb = f32(inputs["in_proj_b"])
    wq, wk, wv = ipw[:E], ipw[E:2 * E], ipw[2 * E:]
    bq, bk_, bv_ = ipb[:E], ipb[E:2 * E], ipb[2 * E:]
    s = 1.0 / np.sqrt(np.float32(D))
    shared = {
        "wqT": bf(wq.T * s), "wkT": bf(wk.T), "wvT": bf(wv.T),
        "woT": bf(f32(inputs["out_proj_w"]).T),
        "w1qT": bf(f32(inputs["mlpq_w1"]).T),
        "w2qT": bf(f32(inputs["mlpq_w2"]).T),
        "w1T": bf(f32(inputs["mlp_w1"]).T),
        "w2T": bf(f32(inputs["mlp_w2"]).T),
        "bqs": f32(bq * s), "bk": f32(bk_), "bv": f32(bv_),
        "bo": f32(inputs["out_proj_b"]),
        "b1q": f32(inputs["mlpq_b1"]), "b2q": f32(inputs["mlpq_b2"]),
        "b1": f32(inputs["mlp_b1"]), "b2": f32(inputs["mlp_b2"]),
        "gq": f32(inputs["ln_q_g"]), "bbq": f32(inputs["ln_q_b"]),
        "gkv": f32(inputs["ln_kv_g"]), "bbkv": f32(inputs["ln_kv_b"]),
        "gres": f32(inputs["res_ln_g"]), "bbres": f32(inputs["res_ln_b"]),
        "gln2": f32(inputs["ln2_g"]), "bbln2": f32(inputs["ln2_b"]),
    }
    return [dict(shared, x=np.ascontiguousarray(x[b].reshape(NTOK, E)))
            for b in range(B)]


def _run(inputs, trace=False):
    from concourse.bass_utils import run_bass_kernel_spmd
    nc = _get_nc()
    in_maps = _prep_in_maps(inputs)
    res = run_bass_kernel_spmd(nc, in_maps, core_ids=list(range(B)),
                               trace=trace)
    outs = np.stack([r["out"].reshape(T, NF, E) for r in res.results])
    return outs, res


def kernel(**inputs) -> np.ndarray:
    outs, _ = _run(inputs, trace=False)
    return outs
